# revision 31
# baseline (speedup 1.0000x reference)
"""Trainium2 Bass kernel for DampedAttention.

Full inputs in, full output out. Sharding: 8 cores = 2 batches x 4 head-groups
(4 heads of dim 64 each per core). Per core:

  QT/KT  [c, s] transposed projections (c on partitions), scale 1/8 folded
         into wq/bq on host; bias folded via per-partition activation bias
         on the PSUM->SBUF copy (ScalarE, idle during projections)
  V      [s, c] natural projection (lhsT for the P@V matmul); bias via a
         K=1 ones-row matmul
  LV     banded 0.4*L^T term precomputed per (hp, qb) into SBUF during the
         projection phase; both heads per matmul (lhsT = [V_h0|V_h1], M=128)
  ST     scores transposed [k, q] per (k-chunk, q-block) so exp(ST) is the
         lhsT-layout P^T needed by P@V -- no on-chip transposes
  exp    software-pipelined: scores(kc+2) and exp(kc+1) run ahead of pv(kc)
         so ScalarE (the bottleneck) streams exps back-to-back
  ctxT   [65, q] = V_aug^T @ P^T ; row 64 = softmax row-sums (ones column)
  blend  ctxT_final = PV * (0.6/r) + 0.4LV; 1/r via DVE reciprocal_approx
         + gpsimd partition broadcast (keeps ScalarE exp-only: one act table)
  out    [s, o] out-projection matmuls injected into later attention groups'
         loops as PE filler; host sums 4 head-group partials + bo

Matmul operands are bf16; accumulation, row-sums, reciprocal and the 0.6/r
normalization stay fp32. The entropy gate in the reference is a forward
no-op and is skipped. Softmax max-subtraction is skipped (scores are O(1)).
"""
import numpy as np
import ml_dtypes

S = 2048
D = 1024
CLOC = 256          # channels per core (4 heads x 64)
HD = 64
NH = 4              # heads per core
NDC = 8             # 128-wide d-chunks in contraction D
NKC = 16            # 128-wide k/s chunks in S
NQB = 4             # 512-wide q blocks
QB = 512
WINDOW = 3
STRENGTH = 0.4
EPS = 1e-10
F32 = np.float32
BF16 = ml_dtypes.bfloat16


def _build_L04T():
    i = np.arange(S)
    d = (i[:, None] - i[None, :]).astype(F32)
    k = np.where(np.abs(d) <= WINDOW,
                 np.exp(-(d ** 2) / F32(2.0 * STRENGTH ** 2)),
                 F32(0.0)).astype(F32)
    L = k / (k.sum(axis=-1, keepdims=True) + F32(EPS))
    return (F32(0.4) * L).T.copy()  # [s, q], pre-scaled by (1 - lambda_jump)


def _lt_tiles():
    """Unique [128, 512] band tiles of 0.4*L^T plus (qb -> [(j, uniq_idx)])."""
    L04T = _build_L04T()
    uniq = []
    slots = {qb: [] for qb in range(NQB)}
    for qb in range(NQB):
        for j in range(max(0, qb * 4 - 1), min(NKC, qb * 4 + 5)):
            t = L04T[j * 128:(j + 1) * 128, qb * QB:(qb + 1) * QB]
            for ui, ut in enumerate(uniq):
                if np.array_equal(t, ut):
                    slots[qb].append((j, ui))
                    break
            else:
                slots[qb].append((j, len(uniq)))
                uniq.append(t)
    return np.stack(uniq).astype(BF16), slots


_LT_UNIQ, _LT_SLOTS = _lt_tiles()
NU = _LT_UNIQ.shape[0]

_CACHE = {}


def _build_program():
    import concourse.bacc as bacc
    import concourse.mybir as mybir
    from concourse.tile import TileContext

    f32 = mybir.dt.float32
    bf16 = mybir.dt.bfloat16
    Exp = mybir.ActivationFunctionType.Exp
    Ident = mybir.ActivationFunctionType.Identity
    mult = mybir.AluOpType.mult
    add = mybir.AluOpType.add

    nc = bacc.Bacc("TRN2", target_bir_lowering=False, debug=False,
                   enable_asserts=False, num_devices=8)

    xt = nc.dram_tensor("xt", [D, S], bf16, kind="ExternalInput").ap()
    wqt = nc.dram_tensor("wqt", [D, CLOC], bf16, kind="ExternalInput").ap()
    wkt = nc.dram_tensor("wkt", [D, CLOC], bf16, kind="ExternalInput").ap()
    wvt = nc.dram_tensor("wvt", [D, CLOC], bf16, kind="ExternalInput").ap()
    bqc = nc.dram_tensor("bqc", [128, 2], f32, kind="ExternalInput").ap()
    bkc = nc.dram_tensor("bkc", [128, 2], f32, kind="ExternalInput").ap()
    bvr = nc.dram_tensor("bvr", [1, CLOC], bf16, kind="ExternalInput").ap()
    wot = nc.dram_tensor("wot", [CLOC, D], bf16, kind="ExternalInput").ap()
    ltt = nc.dram_tensor("ltt", [NU, 128, QB], bf16, kind="ExternalInput").ap()
    out = nc.dram_tensor("out", [S, D], f32, kind="ExternalOutput").ap()

    with TileContext(nc) as tc:
        with tc.tile_pool(name="persist", bufs=1) as pp:
            # ---- persistent SBUF ----
            qt = [pp.tile([128, S], bf16, name=f"qt{i}") for i in range(2)]
            kt = [pp.tile([128, S], bf16, name=f"kt{i}") for i in range(2)]
            v_all = pp.tile([128, NKC, NH, HD + 1], bf16)  # ones col at 64
            v_pair = pp.tile([128, NKC, CLOC], bf16)  # contiguous, no ones col
            lv_sb = pp.tile([128, 2, S], bf16)   # rows 0-63 hh0 / 64-127 hh1
            lv1_sb = pp.tile([64, 2, S], bf16)   # hh1 band term at rows 0-63
            ctxt_all = pp.tile([128, 2, S], bf16)
            wot_sb = pp.tile([128, 2, D], bf16)
            bq_sb = pp.tile([128, 2], f32)       # per-partition bias columns
            bk_sb = pp.tile([128, 2], f32)
            bv_sb = pp.tile([1, CLOC], bf16)
            lt_sb = pp.tile([128, NU, QB], bf16)
            ones_c = pp.tile([1, 128], bf16)     # ones row (V bias)

            nc.gpsimd.memset(ones_c[:], 1.0)
            nc.gpsimd.memset(v_all[:, :, :, HD:HD + 1], 1.0)

            nc.gpsimd.dma_start(bq_sb[:], bqc[:])
            nc.gpsimd.dma_start(bk_sb[:], bkc[:])
            nc.gpsimd.dma_start(bv_sb[:], bvr[:])

            # ---- phase B: projections + LV ----
            with (
                tc.tile_pool(name="projsb", bufs=1) as prs,
                tc.tile_pool(name="projps", bufs=4, space="PSUM") as prp,
                tc.tile_pool(name="vps", bufs=4, space="PSUM") as vpp,
            ):
                xt_sb = prs.tile([128, NDC, S], bf16)
                wq_sb = prs.tile([128, NDC, CLOC], bf16)
                wk_sb = prs.tile([128, NDC, CLOC], bf16)
                wv_sb = prs.tile([128, NDC, CLOC], bf16)
                # DMA issue occupies the issuing engine ~0.6ns/descriptor+
                # ~5ns/KB, so spread the 6MB of loads across the three
                # DMA-capable engines, in consumption order per engine.
                for dc in range(NDC):  # wq/wk dc-granular: matmul dc=0 early
                    nc.scalar.dma_start(wq_sb[:, dc, :],
                                        wqt[dc * 128:(dc + 1) * 128, :])
                    nc.scalar.dma_start(wk_sb[:, dc, :],
                                        wkt[dc * 128:(dc + 1) * 128, :])
                for dc in range(0, NDC, 2):
                    nc.sync.dma_start(xt_sb[:, dc, :],
                                      xt[dc * 128:(dc + 1) * 128, :])
                for dc in range(1, NDC, 2):
                    nc.gpsimd.dma_start(xt_sb[:, dc, :],
                                        xt[dc * 128:(dc + 1) * 128, :])
                for dc in range(NDC):
                    nc.sync.dma_start(wv_sb[:, dc, :],
                                      wvt[dc * 128:(dc + 1) * 128, :])
                nc.gpsimd.dma_start(wot_sb[:, :, :],
                                    wot.rearrange("(cc p) o -> p cc o", p=128))
                nc.gpsimd.dma_start(lt_sb[:, :, :],
                                    ltt.rearrange("u p q -> p u q"))

                # QT/KT: [c-tile 128, s-block 512], contraction over d.
                # dc outermost so one weight load serves 4 qb matmuls; bias
                # added on the PSUM->SBUF copy (ScalarE, per-partition bias).
                for ct in range(2):
                    for dst, w_sb, b_sb in ((qt[ct], wq_sb, bq_sb),
                                            (kt[ct], wk_sb, bk_sb)):
                        pss = [prp.tile([128, QB], f32, tag="projps",
                                        name=f"pjps{qb}") for qb in range(NQB)]
                        for dc in range(NDC):
                            for qb in range(NQB):
                                nc.tensor.matmul(
                                    pss[qb][:],
                                    w_sb[:, dc, ct * 128:(ct + 1) * 128],
                                    xt_sb[:, dc, qb * QB:(qb + 1) * QB],
                                    start=(dc == 0), stop=(dc == NDC - 1))
                        for qb in range(NQB):
                            nc.scalar.activation(
                                dst[:, qb * QB:(qb + 1) * QB], pss[qb][:],
                                Ident, bias=b_sb[:, ct:ct + 1])

                # V natural: [s-chunk 128, 256], contraction over d
                for sc in range(NKC):
                    ps = vpp.tile([128, CLOC], f32, tag="vps")
                    for dc in range(NDC):
                        nc.tensor.matmul(
                            ps[:],
                            xt_sb[:, dc, sc * 128:(sc + 1) * 128],
                            wv_sb[:, dc, :],
                            start=(dc == 0), stop=False)
                    nc.tensor.matmul(ps[:], ones_c[:], bv_sb[:],
                                     start=False, stop=True)
                    nc.vector.tensor_copy(
                        v_all[:, sc, :, 0:HD],
                        ps[:].rearrange("p (h e) -> p h e", h=NH))
                    nc.vector.tensor_copy(v_pair[:, sc, :], ps[:])



            # ---- phase C: attention, software-pipelined ----
            # Heads 2hp/2hp+1 live at partitions 0-63/64-127 of c-tile hp.
            # Per (qb, hp) group: scores(kc+2)/exp(kc+1) run ahead of pv(kc)
            # so ScalarE (exp, the bottleneck) streams back-to-back while PE
            # fills its spare cycles with injected out-projection matmuls.
            with (
                tc.tile_pool(name="pt", bufs=6) as ptp,
                tc.tile_pool(name="stage", bufs=4) as sp,
                tc.tile_pool(name="osb", bufs=4) as ob,
                tc.tile_pool(name="stps", bufs=2, space="PSUM") as stp,
                tc.tile_pool(name="ctxps", bufs=3, space="PSUM") as ctp,
            ):
                def emit_outproj(sc, ot, pool, sbpool):
                    ps = pool.tile([128, QB], f32, tag="ops")
                    # cc=1 first: hp1's ctxt half is blended later, so the
                    # scheduler cannot hoist this pair to a point where it
                    # would stall PE waiting on the hp0 stg DMA.
                    for cc in (1, 0):
                        nc.tensor.matmul(
                            ps[:],
                            ctxt_all[:, cc, sc * 128:(sc + 1) * 128],
                            wot_sb[:, cc, ot * QB:(ot + 1) * QB],
                            start=(cc == 1), stop=(cc == 0),
                            skip_group_check=True)
                    o_sb = sbpool.tile([128, QB], f32, tag="osb")
                    nc.vector.tensor_copy(o_sb[:], ps[:])
                    nc.sync.dma_start(
                        out[sc * 128:(sc + 1) * 128, ot * QB:(ot + 1) * QB],
                        o_sb[:])

                def lv_closures(lvp):
                    # banded 0.4*L^T @ V, both heads per matmul (M=128),
                    # chopped into per-matmul closures injected into the
                    # first two groups' kc loops as PE filler.
                    items = []
                    for qb, hp in groups:
                        qsl = slice(qb * QB, (qb + 1) * QB)
                        slots = _LT_SLOTS[qb]
                        cell = {}

                        def mk(n, j, u, qb, hp, qsl, cell, last):
                            def go():
                                if n == 0:
                                    cell['ps'] = lvp.tile(
                                        [128, QB], f32, tag="lvps",
                                        name="lv_ps")
                                nc.tensor.matmul(
                                    cell['ps'][:],
                                    v_pair[:, j, hp * 128:(hp + 1) * 128],
                                    lt_sb[:, u, :],
                                    start=(n == 0), stop=last,
                                    skip_group_check=True)
                                if last:
                                    nc.vector.tensor_copy(
                                        lv_sb[:, hp, qsl], cell['ps'][:])
                                    nc.sync.dma_start(
                                        lv1_sb[:, hp, qsl],
                                        lv_sb[64:128, hp, qsl])
                            return go
                        for n, (j, u) in enumerate(slots):
                            items.append(mk(n, j, u, qb, hp, qsl, cell,
                                            n == len(slots) - 1))
                    return items

                filler = []

                def mk_blend(qb, hp, css):
                    # blend: ctxt = (PV * 0.6/r) + 0.4LV, running entirely
                    # off the SBUF cs copies. Deferred into the NEXT group's
                    # kc loop so nothing here gates a group boundary.
                    qsl = slice(qb * QB, (qb + 1) * QB)

                    def blend():
                        for hh in range(2):
                            cs = css[hh]
                            # partition 64 -> 0 move on ScalarE (tiny; Copy
                            # needs no table so the Exp table stays resident)
                            bcs = sp.tile([1, QB], f32, tag="bcs")
                            nc.scalar.activation(
                                bcs[0:1, :], cs[64:65, :],
                                mybir.ActivationFunctionType.Copy)
                            rc = sp.tile([1, QB], f32, tag="rc")
                            nc.vector.reciprocal_approx_fast(rc[:], bcs[:])
                            bc = sp.tile([64, QB], f32, tag="bc")
                            nc.gpsimd.partition_broadcast(
                                bc[:], rc[0:1, :], channels=HD)
                            m1 = sp.tile([64, QB], f32, tag="m1")
                            nc.vector.tensor_mul(m1[:], cs[0:HD, :], bc[:])
                            if hh == 0:
                                nc.vector.scalar_tensor_tensor(
                                    ctxt_all[0:64, hp, qsl], m1[:], 0.6,
                                    lv_sb[0:64, hp, qsl], op0=mult, op1=add)
                            else:
                                stg = sp.tile([64, QB], bf16, tag="stg")
                                nc.vector.scalar_tensor_tensor(
                                    stg[:], m1[:], 0.6,
                                    lv1_sb[:, hp, qsl], op0=mult, op1=add)
                                nc.sync.dma_start(
                                    ctxt_all[64:128, hp, qsl], stg[:])
                    return blend

                def emit_group(qb, hp, prev_blend, pops):
                    qsl = slice(qb * QB, (qb + 1) * QB)
                    ctx = [ctp.tile([128, QB], f32, tag="ctxps",
                                    name=f"ctx{hh}") for hh in range(2)]
                    sts, pts = {}, {}

                    def scores(kc):
                        st = stp.tile([128, 2, QB], f32, tag="stps")
                        sts[kc] = st
                        for hh in range(2):
                            p0 = hh * 64
                            nc.tensor.matmul(
                                st[:, hh, :],
                                kt[hp][p0:p0 + 64, kc * 128:(kc + 1) * 128],
                                qt[hp][p0:p0 + 64, qsl],
                                start=True, stop=True,
                                tile_position=(p0, 0))
                        ptt = ptp.tile([128, 2, QB], bf16, tag="pt")
                        pts[kc] = ptt
                        nc.scalar.activation(ptt[:], st[:], Exp)

                    def pv(kc):
                        ptt = pts.pop(kc)
                        sts.pop(kc)
                        for hh in range(2):
                            nc.tensor.matmul(
                                ctx[hh][0:HD + 1, :],
                                v_all[:, kc, 2 * hp + hh, 0:HD + 1],
                                ptt[:, hh, :],
                                start=(kc == 0), stop=(kc == NKC - 1))

                    scores(0)
                    scores(1)
                    for kc in range(NKC):
                        if kc + 2 < NKC:
                            scores(kc + 2)
                        pv(kc)
                        if kc == 1 and prev_blend is not None:
                            prev_blend()
                        for _ in range(pops(kc)):
                            if filler:
                                filler.pop(0)()
                    # copy ctx off PSUM right away: the banks free ~1us after
                    # pv(15), so the next group's accumulation never stalls
                    css = []
                    for hh in range(2):
                        cs = sp.tile([65, QB], f32, tag="cs")
                        nc.vector.tensor_copy(cs[:], ctx[hh][0:HD + 1, :])
                        css.append(cs)
                    return mk_blend(qb, hp, css)

                # (qb, hp) order chosen so out-proj(qb) becomes injectable
                # 1-2 groups after both hp halves of qb have blended.
                groups = [(0, 0), (0, 1), (1, 0), (1, 1),
                          (2, 0), (2, 1), (3, 0), (3, 1)]
                lv_pops = lambda kc: 2
                op_pops = lambda kc: 1 if kc in (5, 8, 11, 14) else 0
                push_after = {2: 0, 4: 1, 6: 2}
                pending_blend = None
                # g0/g1: LV matmuls (PE filler) under a 1-bank PSUM pool that
                # closes before the out-projection pool opens.
                with tc.tile_pool(name="lvps", bufs=1, space="PSUM") as lvp:
                    filler.extend(lv_closures(lvp))
                    for gi in (0, 1):
                        qb, hp = groups[gi]
                        pending_blend = emit_group(qb, hp, pending_blend,
                                                   lv_pops)
                    while filler:  # safety: drain any LV leftovers
                        filler.pop(0)()
                opq = [(sc, ot) for qb_o in range(3)
                       for ot in range(2) for sc in range(qb_o * 4, qb_o * 4 + 4)]
                op_state = {"pushed": 0, "popped": 0}
                with tc.tile_pool(name="ops", bufs=1, space="PSUM") as opp:
                    def op_closure():
                        if op_state["popped"] < op_state["pushed"]:
                            sc, ot = opq[op_state["popped"]]
                            op_state["popped"] += 1
                            emit_outproj(sc, ot, opp, ob)
                    filler.extend([op_closure] * 64)  # pops gated by pushed
                    for gi in range(2, 8):
                        qb, hp = groups[gi]
                        pending_blend = emit_group(qb, hp, pending_blend,
                                                   op_pops)
                        if gi in push_after:
                            op_state["pushed"] = 8 * (push_after[gi] + 1)
                    pending_blend()
                    leftovers = opq[op_state["popped"]:]
                    filler.clear()

            # ---- phase D: out-projection tail (wider PSUM pool) ----
            with (
                tc.tile_pool(name="ops2", bufs=4, space="PSUM") as opp2,
                tc.tile_pool(name="osb2", bufs=4) as ob2,
            ):
                for sc, ot in leftovers:
                    emit_outproj(sc, ot, opp2, ob2)
                for sc in range(12, 16):
                    for ot in range(2):
                        emit_outproj(sc, ot, opp2, ob2)

    nc.compile()
    return nc


def _get_program():
    if "nc" not in _CACHE:
        _CACHE["nc"] = _build_program()
    return _CACHE["nc"]


def _in_maps(x, Wq, bq, Wk, bk, Wv, bv, Wo):
    xT = [np.ascontiguousarray(x[b].T).astype(BF16) for b in range(2)]
    maps = []
    for c in range(8):
        b, hg = c // 4, c % 4
        hs, he = hg * CLOC, (hg + 1) * CLOC
        maps.append({
            "xt": xT[b],
            "wqt": np.ascontiguousarray(Wq[hs:he].T / F32(8.0)).astype(BF16),
            "wkt": np.ascontiguousarray(Wk[hs:he].T).astype(BF16),
            "wvt": np.ascontiguousarray(Wv[hs:he].T).astype(BF16),
            "bqc": np.ascontiguousarray((bq[hs:he] / F32(8.0)).reshape(2, 128).T),
            "bkc": np.ascontiguousarray(bk[hs:he].reshape(2, 128).T),
            "bvr": bv[hs:he][None, :].astype(BF16),
            "wot": np.ascontiguousarray(Wo[:, hs:he].T).astype(BF16),
            "ltt": _LT_UNIQ,
        })
    return maps


def _run(x, Wq, bq, Wk, bk, Wv, bv, Wo, bo, trace=False):
    from concourse.bass_utils import run_bass_kernel_spmd
    nc = _get_program()
    maps = _in_maps(np.asarray(x, F32), np.asarray(Wq, F32), np.asarray(bq, F32),
                    np.asarray(Wk, F32), np.asarray(bk, F32), np.asarray(Wv, F32),
                    np.asarray(bv, F32), np.asarray(Wo, F32))
    res = run_bass_kernel_spmd(nc, maps, list(range(8)), trace=trace)
    bo = np.asarray(bo, F32)
    outp = np.empty((2, S, D), F32)
    for b in range(2):
        acc = res.results[b * 4]["out"].astype(F32)
        for hg in range(1, 4):
            acc = acc + res.results[b * 4 + hg]["out"]
        outp[b] = acc + bo
    return outp, res


def kernel(x, Wq, bq, Wk, bk, Wv, bv, Wo, bo):
    outp, _ = _run(x, Wq, bq, Wk, bk, Wv, bv, Wo, bo, trace=False)
    return outp


def kernel_traced(**inputs):
    return _run(trace=True, **inputs)


# revision 37
# speedup vs baseline: 1.0293x; 1.0293x over previous
"""Trainium2 Bass kernel for DampedAttention.

Full inputs in, full output out. Sharding: 8 cores = 2 batches x 4 head-groups
(4 heads of dim 64 each per core). Per core:

  QT/KT  [c, s] transposed projections (c on partitions), scale 1/8 folded
         into wq/bq on host; bias folded via per-partition activation bias
         on the PSUM->SBUF copy (ScalarE, idle during projections)
  V      [s, c] natural projection (lhsT for the P@V matmul); bias via a
         K=1 ones-row matmul
  LV     banded 0.4*L^T term precomputed per (hp, qb) into SBUF during the
         projection phase; both heads per matmul (lhsT = [V_h0|V_h1], M=128)
  ST     scores transposed [k, q] per (k-chunk, q-block) so exp(ST) is the
         lhsT-layout P^T needed by P@V -- no on-chip transposes
  exp    software-pipelined: scores(kc+2) and exp(kc+1) run ahead of pv(kc)
         so ScalarE (the bottleneck) streams exps back-to-back
  ctxT   [65, q] = V_aug^T @ P^T ; row 64 = softmax row-sums (ones column)
  blend  ctxT_final = PV * (0.6/r) + 0.4LV; 1/r via DVE reciprocal_approx
         + gpsimd partition broadcast (keeps ScalarE exp-only: one act table)
  out    [s, o] out-projection matmuls injected into later attention groups'
         loops as PE filler; host sums 4 head-group partials + bo

Matmul operands are bf16; accumulation, row-sums, reciprocal and the 0.6/r
normalization stay fp32. The entropy gate in the reference is a forward
no-op and is skipped. Softmax max-subtraction is skipped (scores are O(1)).
"""
import numpy as np
import ml_dtypes

S = 2048
D = 1024
CLOC = 256          # channels per core (4 heads x 64)
HD = 64
NH = 4              # heads per core
NDC = 8             # 128-wide d-chunks in contraction D
NKC = 16            # 128-wide k/s chunks in S
NQB = 4             # 512-wide q blocks
QB = 512
WINDOW = 3
STRENGTH = 0.4
EPS = 1e-10
F32 = np.float32
BF16 = ml_dtypes.bfloat16


def _build_L04T():
    i = np.arange(S)
    d = (i[:, None] - i[None, :]).astype(F32)
    k = np.where(np.abs(d) <= WINDOW,
                 np.exp(-(d ** 2) / F32(2.0 * STRENGTH ** 2)),
                 F32(0.0)).astype(F32)
    L = k / (k.sum(axis=-1, keepdims=True) + F32(EPS))
    return (F32(0.4) * L).T.copy()  # [s, q], pre-scaled by (1 - lambda_jump)


def _lt_tiles():
    """Unique [128, 512] band tiles of 0.4*L^T plus (qb -> [(j, uniq_idx)])."""
    L04T = _build_L04T()
    uniq = []
    slots = {qb: [] for qb in range(NQB)}
    for qb in range(NQB):
        for j in range(max(0, qb * 4 - 1), min(NKC, qb * 4 + 5)):
            t = L04T[j * 128:(j + 1) * 128, qb * QB:(qb + 1) * QB]
            for ui, ut in enumerate(uniq):
                if np.array_equal(t, ut):
                    slots[qb].append((j, ui))
                    break
            else:
                slots[qb].append((j, len(uniq)))
                uniq.append(t)
    return np.stack(uniq).astype(BF16), slots


_LT_UNIQ, _LT_SLOTS = _lt_tiles()
NU = _LT_UNIQ.shape[0]

_CACHE = {}


def _build_program():
    import concourse.bacc as bacc
    import concourse.mybir as mybir
    from concourse.tile import TileContext

    f32 = mybir.dt.float32
    bf16 = mybir.dt.bfloat16
    Exp = mybir.ActivationFunctionType.Exp
    Ident = mybir.ActivationFunctionType.Identity
    mult = mybir.AluOpType.mult
    add = mybir.AluOpType.add

    nc = bacc.Bacc("TRN2", target_bir_lowering=False, debug=False,
                   enable_asserts=False, num_devices=8)

    xt = nc.dram_tensor("xt", [D, S], bf16, kind="ExternalInput").ap()
    wqt = nc.dram_tensor("wqt", [D, CLOC], bf16, kind="ExternalInput").ap()
    wkt = nc.dram_tensor("wkt", [D, CLOC], bf16, kind="ExternalInput").ap()
    wvt = nc.dram_tensor("wvt", [D, CLOC], bf16, kind="ExternalInput").ap()
    bqc = nc.dram_tensor("bqc", [128, 2], f32, kind="ExternalInput").ap()
    bkc = nc.dram_tensor("bkc", [128, 2], f32, kind="ExternalInput").ap()
    bvr = nc.dram_tensor("bvr", [1, CLOC], bf16, kind="ExternalInput").ap()
    wot = nc.dram_tensor("wot", [CLOC, D], bf16, kind="ExternalInput").ap()
    ltt = nc.dram_tensor("ltt", [NU, 128, QB], bf16, kind="ExternalInput").ap()
    out = nc.dram_tensor("out", [S, D], f32, kind="ExternalOutput").ap()

    with TileContext(nc) as tc:
        with tc.tile_pool(name="persist", bufs=1) as pp:
            # ---- persistent SBUF ----
            qt = [pp.tile([128, S], bf16, name=f"qt{i}") for i in range(2)]
            kt = [pp.tile([128, S], bf16, name=f"kt{i}") for i in range(2)]
            v_all = pp.tile([128, NKC, NH, HD + 1], bf16)  # 1/0.6 col at 64
            v_pair = pp.tile([128, NKC, CLOC], bf16)  # contiguous, no ones col
            ctxt_all = pp.tile([128, 2, S], bf16)
            wot_sb = pp.tile([128, 2, D], bf16)
            bq_sb = pp.tile([128, 2], f32)       # per-partition bias columns
            bk_sb = pp.tile([128, 2], f32)
            bv_sb = pp.tile([1, CLOC], bf16)
            lt_sb = pp.tile([128, NU, QB], bf16)
            ones_c = pp.tile([1, 128], bf16)     # ones row (V bias)

            nc.gpsimd.memset(ones_c[:], 1.0)
            # rowsum column pre-scaled by 1/lambda_jump so 1/rowsum' = 0.6/r
            nc.gpsimd.memset(v_all[:, :, :, HD:HD + 1], 1.0 / 0.6)

            nc.gpsimd.dma_start(bq_sb[:], bqc[:])
            nc.gpsimd.dma_start(bk_sb[:], bkc[:])
            nc.gpsimd.dma_start(bv_sb[:], bvr[:])

            # ---- phase B: projections + LV ----
            with (
                tc.tile_pool(name="projsb", bufs=1) as prs,
                tc.tile_pool(name="projps", bufs=4, space="PSUM") as prp,
                tc.tile_pool(name="vps", bufs=4, space="PSUM") as vpp,
            ):
                xt_sb = prs.tile([128, NDC, S], bf16)
                wq_sb = prs.tile([128, NDC, CLOC], bf16)
                wk_sb = prs.tile([128, NDC, CLOC], bf16)
                wv_sb = prs.tile([128, NDC, CLOC], bf16)
                # DMA issue occupies the issuing engine ~0.6ns/descriptor+
                # ~5ns/KB, so spread the 6MB of loads across the three
                # DMA-capable engines, in consumption order per engine.
                for dc in range(NDC):  # wq/wk dc-granular: matmul dc=0 early
                    nc.scalar.dma_start(wq_sb[:, dc, :],
                                        wqt[dc * 128:(dc + 1) * 128, :])
                    nc.scalar.dma_start(wk_sb[:, dc, :],
                                        wkt[dc * 128:(dc + 1) * 128, :])
                for dc in range(0, NDC, 2):
                    nc.sync.dma_start(xt_sb[:, dc, :],
                                      xt[dc * 128:(dc + 1) * 128, :])
                for dc in range(1, NDC, 2):
                    nc.gpsimd.dma_start(xt_sb[:, dc, :],
                                        xt[dc * 128:(dc + 1) * 128, :])
                for dc in range(NDC):
                    nc.sync.dma_start(wv_sb[:, dc, :],
                                      wvt[dc * 128:(dc + 1) * 128, :])
                nc.gpsimd.dma_start(wot_sb[:, :, :],
                                    wot.rearrange("(cc p) o -> p cc o", p=128))
                nc.gpsimd.dma_start(lt_sb[:, :, :],
                                    ltt.rearrange("u p q -> p u q"))

                # QT/KT: [c-tile 128, s-block 512], contraction over d.
                # dc outermost so one weight load serves 4 qb matmuls; bias
                # added on the PSUM->SBUF copy (ScalarE, per-partition bias).
                for ct in range(2):
                    for dst, w_sb, b_sb in ((qt[ct], wq_sb, bq_sb),
                                            (kt[ct], wk_sb, bk_sb)):
                        pss = [prp.tile([128, QB], f32, tag="projps",
                                        name=f"pjps{qb}") for qb in range(NQB)]
                        for dc in range(NDC):
                            for qb in range(NQB):
                                nc.tensor.matmul(
                                    pss[qb][:],
                                    w_sb[:, dc, ct * 128:(ct + 1) * 128],
                                    xt_sb[:, dc, qb * QB:(qb + 1) * QB],
                                    start=(dc == 0), stop=(dc == NDC - 1))
                        for qb in range(NQB):
                            nc.scalar.activation(
                                dst[:, qb * QB:(qb + 1) * QB], pss[qb][:],
                                Ident, bias=b_sb[:, ct:ct + 1])

                # V natural: [s-chunk 128, 256], contraction over d
                for sc in range(NKC):
                    ps = vpp.tile([128, CLOC], f32, tag="vps")
                    for dc in range(NDC):
                        nc.tensor.matmul(
                            ps[:],
                            xt_sb[:, dc, sc * 128:(sc + 1) * 128],
                            wv_sb[:, dc, :],
                            start=(dc == 0), stop=False)
                    nc.tensor.matmul(ps[:], ones_c[:], bv_sb[:],
                                     start=False, stop=True)
                    nc.vector.tensor_copy(
                        v_all[:, sc, :, 0:HD],
                        ps[:].rearrange("p (h e) -> p h e", h=NH))
                    nc.vector.tensor_copy(v_pair[:, sc, :], ps[:])



            # ---- phase C: attention, software-pipelined ----
            # Heads 2hp/2hp+1 live at partitions 0-63/64-127 of c-tile hp.
            # Per (qb, hp) group: scores(kc+2)/exp(kc+1) run ahead of pv(kc)
            # so ScalarE (exp, the bottleneck) streams back-to-back while PE
            # fills its spare cycles with injected out-projection matmuls.
            with (
                tc.tile_pool(name="pt", bufs=6) as ptp,
                tc.tile_pool(name="stage", bufs=4) as sp,
                tc.tile_pool(name="osb", bufs=4) as ob,
                tc.tile_pool(name="stps", bufs=2, space="PSUM") as stp,
                tc.tile_pool(name="ctxps", bufs=3, space="PSUM") as ctp,
            ):
                def emit_outproj(sc, ot, pool, sbpool):
                    ps = pool.tile([128, QB], f32, tag="ops")
                    # cc=1 first: hp1's ctxt half is blended later, so the
                    # scheduler cannot hoist this pair to a point where it
                    # would stall PE waiting on the hp0 stg DMA.
                    for cc in (1, 0):
                        nc.tensor.matmul(
                            ps[:],
                            ctxt_all[:, cc, sc * 128:(sc + 1) * 128],
                            wot_sb[:, cc, ot * QB:(ot + 1) * QB],
                            start=(cc == 1), stop=(cc == 0),
                            skip_group_check=True)
                    o_sb = sbpool.tile([128, QB], f32, tag="osb")
                    nc.vector.tensor_copy(o_sb[:], ps[:])
                    nc.sync.dma_start(
                        out[sc * 128:(sc + 1) * 128, ot * QB:(ot + 1) * QB],
                        o_sb[:])

                def lv_closures(lvp):
                    # banded 0.4*L^T @ V, both heads per matmul (M=128),
                    # chopped into per-matmul closures injected into the
                    # first two groups' kc loops as PE filler.
                    items = []
                    for qb, hp in groups:
                        qsl = slice(qb * QB, (qb + 1) * QB)
                        slots = _LT_SLOTS[qb]
                        cell = {}

                        def mk(n, j, u, qb, hp, qsl, cell, last):
                            def go():
                                if n == 0:
                                    cell['ps'] = lvp.tile(
                                        [128, QB], f32, tag="lvps",
                                        name="lv_ps")
                                nc.tensor.matmul(
                                    cell['ps'][:],
                                    v_pair[:, j, hp * 128:(hp + 1) * 128],
                                    lt_sb[:, u, :],
                                    start=(n == 0), stop=last,
                                    skip_group_check=True)
                                if last:
                                    nc.vector.tensor_copy(
                                        ctxt_all[:, hp, qsl], cell['ps'][:])
                            return go
                        for n, (j, u) in enumerate(slots):
                            items.append(mk(n, j, u, qb, hp, qsl, cell,
                                            n == len(slots) - 1))
                    return items

                filler = []

                def mk_blend(qb, hp, css):
                    # blend: ctxt (pre-loaded with 0.4LV) += PV * 0.6/r,
                    # running entirely off the SBUF cs copies. Deferred into
                    # the NEXT group's kc loop so nothing gates a boundary.
                    # ScalarE is untouched: the partition 64->0 move is a
                    # tiny SBUF-SBUF DMA; the adds are DMA accumulates.
                    qsl = slice(qb * QB, (qb + 1) * QB)

                    def blend():
                        for hh in range(2):
                            cs = css[hh]
                            bcs = sp.tile([1, QB], f32, tag="bcs")
                            nc.sync.dma_start(bcs[0:1, :], cs[64:65, :])
                            rc = sp.tile([1, QB], f32, tag="rc")
                            nc.vector.reciprocal_approx_fast(rc[:], bcs[:])
                            bc = sp.tile([64, QB], f32, tag="bc")
                            nc.gpsimd.partition_broadcast(
                                bc[:], rc[0:1, :], channels=HD)
                            m1 = sp.tile([64, QB], bf16, tag="m1")
                            nc.vector.tensor_mul(m1[:], cs[0:HD, :], bc[:])
                            nc.gpsimd.dma_start(
                                ctxt_all[hh * 64:hh * 64 + 64, hp, qsl],
                                m1[:], accum_op=add)
                    return blend

                def emit_group(qb, hp, prev_blend, pops):
                    qsl = slice(qb * QB, (qb + 1) * QB)
                    ctx = [ctp.tile([128, QB], f32, tag="ctxps",
                                    name=f"ctx{hh}") for hh in range(2)]
                    sts, pts = {}, {}

                    def scores(kc):
                        st = stp.tile([128, 2, QB], f32, tag="stps")
                        sts[kc] = st
                        for hh in range(2):
                            p0 = hh * 64
                            nc.tensor.matmul(
                                st[:, hh, :],
                                kt[hp][p0:p0 + 64, kc * 128:(kc + 1) * 128],
                                qt[hp][p0:p0 + 64, qsl],
                                start=True, stop=True,
                                tile_position=(p0, 0))
                        ptt = ptp.tile([128, 2, QB], bf16, tag="pt")
                        pts[kc] = ptt
                        nc.scalar.activation(ptt[:], st[:], Exp)

                    def pv(kc):
                        ptt = pts.pop(kc)
                        sts.pop(kc)
                        for hh in range(2):
                            nc.tensor.matmul(
                                ctx[hh][0:HD + 1, :],
                                v_all[:, kc, 2 * hp + hh, 0:HD + 1],
                                ptt[:, hh, :],
                                start=(kc == 0), stop=(kc == NKC - 1))

                    scores(0)
                    scores(1)
                    for kc in range(NKC):
                        if kc + 2 < NKC:
                            scores(kc + 2)
                        pv(kc)
                        if kc == 1 and prev_blend is not None:
                            prev_blend()
                        for _ in range(pops(kc)):
                            if filler:
                                filler.pop(0)()
                    # copy ctx off PSUM right away: the banks free ~1us after
                    # pv(15), so the next group's accumulation never stalls
                    css = []
                    for hh in range(2):
                        cs = sp.tile([65, QB], f32, tag="cs")
                        nc.vector.tensor_copy(cs[:], ctx[hh][0:HD + 1, :])
                        css.append(cs)
                    return mk_blend(qb, hp, css)

                # (qb, hp) order chosen so out-proj(qb) becomes injectable
                # 1-2 groups after both hp halves of qb have blended.
                groups = [(0, 0), (0, 1), (1, 0), (1, 1),
                          (2, 0), (2, 1), (3, 0), (3, 1)]
                lv_pops = lambda kc: 2
                op_pops = lambda kc: 1 if kc in (3, 5, 8, 11, 14) else 0
                push_after = {2: 0, 4: 1, 6: 2}
                pending_blend = None
                # g0/g1: LV matmuls (PE filler) under a 1-bank PSUM pool that
                # closes before the out-projection pool opens.
                with tc.tile_pool(name="lvps", bufs=1, space="PSUM") as lvp:
                    filler.extend(lv_closures(lvp))
                    for gi in (0, 1):
                        qb, hp = groups[gi]
                        pending_blend = emit_group(qb, hp, pending_blend,
                                                   lv_pops)
                    while filler:  # safety: drain any LV leftovers
                        filler.pop(0)()
                opq = [(sc, ot) for qb_o in range(3)
                       for ot in range(2) for sc in range(qb_o * 4, qb_o * 4 + 4)]
                op_state = {"pushed": 0, "popped": 0}
                with tc.tile_pool(name="ops", bufs=1, space="PSUM") as opp:
                    def op_closure():
                        if op_state["popped"] < op_state["pushed"]:
                            sc, ot = opq[op_state["popped"]]
                            op_state["popped"] += 1
                            emit_outproj(sc, ot, opp, ob)
                    filler.extend([op_closure] * 64)  # pops gated by pushed
                    for gi in range(2, 8):
                        qb, hp = groups[gi]
                        pending_blend = emit_group(qb, hp, pending_blend,
                                                   op_pops)
                        if gi in push_after:
                            op_state["pushed"] = 8 * (push_after[gi] + 1)
                    pending_blend()
                    leftovers = opq[op_state["popped"]:]
                    filler.clear()

            # ---- phase D: out-projection tail (wider PSUM pool) ----
            with (
                tc.tile_pool(name="ops2", bufs=4, space="PSUM") as opp2,
                tc.tile_pool(name="osb2", bufs=4) as ob2,
            ):
                for sc, ot in leftovers:
                    emit_outproj(sc, ot, opp2, ob2)
                for sc in range(12, 16):
                    for ot in range(2):
                        emit_outproj(sc, ot, opp2, ob2)

    nc.compile()
    return nc


def _get_program():
    if "nc" not in _CACHE:
        _CACHE["nc"] = _build_program()
    return _CACHE["nc"]


def _in_maps(x, Wq, bq, Wk, bk, Wv, bv, Wo):
    xT = [np.ascontiguousarray(x[b].T).astype(BF16) for b in range(2)]
    maps = []
    for c in range(8):
        b, hg = c // 4, c % 4
        hs, he = hg * CLOC, (hg + 1) * CLOC
        maps.append({
            "xt": xT[b],
            "wqt": np.ascontiguousarray(Wq[hs:he].T / F32(8.0)).astype(BF16),
            "wkt": np.ascontiguousarray(Wk[hs:he].T).astype(BF16),
            "wvt": np.ascontiguousarray(Wv[hs:he].T).astype(BF16),
            "bqc": np.ascontiguousarray((bq[hs:he] / F32(8.0)).reshape(2, 128).T),
            "bkc": np.ascontiguousarray(bk[hs:he].reshape(2, 128).T),
            "bvr": bv[hs:he][None, :].astype(BF16),
            "wot": np.ascontiguousarray(Wo[:, hs:he].T).astype(BF16),
            "ltt": _LT_UNIQ,
        })
    return maps


def _run(x, Wq, bq, Wk, bk, Wv, bv, Wo, bo, trace=False):
    from concourse.bass_utils import run_bass_kernel_spmd
    nc = _get_program()
    maps = _in_maps(np.asarray(x, F32), np.asarray(Wq, F32), np.asarray(bq, F32),
                    np.asarray(Wk, F32), np.asarray(bk, F32), np.asarray(Wv, F32),
                    np.asarray(bv, F32), np.asarray(Wo, F32))
    res = run_bass_kernel_spmd(nc, maps, list(range(8)), trace=trace)
    bo = np.asarray(bo, F32)
    outp = np.empty((2, S, D), F32)
    for b in range(2):
        acc = res.results[b * 4]["out"].astype(F32)
        for hg in range(1, 4):
            acc = acc + res.results[b * 4 + hg]["out"]
        outp[b] = acc + bo
    return outp, res


def kernel(x, Wq, bq, Wk, bk, Wv, bv, Wo, bo):
    outp, _ = _run(x, Wq, bq, Wk, bk, Wv, bv, Wo, bo, trace=False)
    return outp


def kernel_traced(**inputs):
    return _run(trace=True, **inputs)


# revision 49
# speedup vs baseline: 1.0572x; 1.0271x over previous
"""Trainium2 Bass kernel for DampedAttention.

Full inputs in, full output out. Sharding: 8 cores = 2 batches x 4 head-groups
(4 heads of dim 64 each per core). Per core:

  QT/KT  [c, s] transposed projections (c on partitions), scale 1/8 folded
         into wq/bq on host; bias folded via per-partition activation bias
         on the PSUM->SBUF copy (ScalarE, idle during projections)
  V      [s, c] natural projection (lhsT for the P@V matmul); bias via a
         K=1 ones-row matmul
  LV     banded 0.4*L^T term precomputed per (hp, qb) into SBUF during the
         projection phase; both heads per matmul (lhsT = [V_h0|V_h1], M=128)
  ST     scores transposed [k, q] per (k-chunk, q-block) so exp(ST) is the
         lhsT-layout P^T needed by P@V -- no on-chip transposes
  exp    software-pipelined: scores(kc+2) and exp(kc+1) run ahead of pv(kc)
         so ScalarE (the bottleneck) streams exps back-to-back
  ctxT   [65, q] = V_aug^T @ P^T ; row 64 = softmax row-sums (ones column)
  blend  ctxT_final = PV * (0.6/r) + 0.4LV; 1/r via DVE reciprocal_approx
         + gpsimd partition broadcast (keeps ScalarE exp-only: one act table)
  out    [s, o] out-projection matmuls injected into later attention groups'
         loops as PE filler; host sums 4 head-group partials + bo

Matmul operands are bf16; accumulation, row-sums, reciprocal and the 0.6/r
normalization stay fp32. The entropy gate in the reference is a forward
no-op and is skipped. Softmax max-subtraction is skipped (scores are O(1)).
"""
import numpy as np
import ml_dtypes

S = 2048
D = 1024
CLOC = 256          # channels per core (4 heads x 64)
HD = 64
NH = 4              # heads per core
NDC = 8             # 128-wide d-chunks in contraction D
NKC = 16            # 128-wide k/s chunks in S
NQB = 4             # 512-wide q blocks
QB = 512
WINDOW = 3
STRENGTH = 0.4
EPS = 1e-10
F32 = np.float32
BF16 = ml_dtypes.bfloat16


def _build_L04T():
    i = np.arange(S)
    d = (i[:, None] - i[None, :]).astype(F32)
    k = np.where(np.abs(d) <= WINDOW,
                 np.exp(-(d ** 2) / F32(2.0 * STRENGTH ** 2)),
                 F32(0.0)).astype(F32)
    L = k / (k.sum(axis=-1, keepdims=True) + F32(EPS))
    return (F32(0.4) * L).T.copy()  # [s, q], pre-scaled by (1 - lambda_jump)


def _lt_tiles():
    """Unique [128, 512] band tiles of 0.4*L^T plus (qb -> [(j, uniq_idx)])."""
    L04T = _build_L04T()
    uniq = []
    slots = {qb: [] for qb in range(NQB)}
    for qb in range(NQB):
        for j in range(max(0, qb * 4 - 1), min(NKC, qb * 4 + 5)):
            t = L04T[j * 128:(j + 1) * 128, qb * QB:(qb + 1) * QB]
            for ui, ut in enumerate(uniq):
                if np.array_equal(t, ut):
                    slots[qb].append((j, ui))
                    break
            else:
                slots[qb].append((j, len(uniq)))
                uniq.append(t)
    return np.stack(uniq).astype(BF16), slots


_LT_UNIQ, _LT_SLOTS = _lt_tiles()
NU = _LT_UNIQ.shape[0]

_CACHE = {}


def _build_program():
    import concourse.bacc as bacc
    import concourse.mybir as mybir
    from concourse.tile import TileContext

    f32 = mybir.dt.float32
    bf16 = mybir.dt.bfloat16
    fp8 = mybir.dt.float8e4
    Exp = mybir.ActivationFunctionType.Exp
    Ident = mybir.ActivationFunctionType.Identity
    mult = mybir.AluOpType.mult
    add = mybir.AluOpType.add
    DR = mybir.MatmulPerfMode.DoubleRow

    nc = bacc.Bacc("TRN2", target_bir_lowering=False, debug=False,
                   enable_asserts=False, num_devices=8)

    xt = nc.dram_tensor("xt", [D, S], bf16, kind="ExternalInput").ap()
    xt8 = nc.dram_tensor("xt8", [D, S], fp8, kind="ExternalInput").ap()
    wq8 = nc.dram_tensor("wq8", [D, CLOC], fp8, kind="ExternalInput").ap()
    wk8 = nc.dram_tensor("wk8", [D, CLOC], fp8, kind="ExternalInput").ap()
    wvt = nc.dram_tensor("wvt", [D, CLOC], bf16, kind="ExternalInput").ap()
    bqc = nc.dram_tensor("bqc", [128, 2], f32, kind="ExternalInput").ap()
    bkc = nc.dram_tensor("bkc", [128, 2], f32, kind="ExternalInput").ap()
    bvr = nc.dram_tensor("bvr", [1, CLOC], bf16, kind="ExternalInput").ap()
    wot = nc.dram_tensor("wot", [CLOC, D], bf16, kind="ExternalInput").ap()
    ltt = nc.dram_tensor("ltt", [NU, 128, QB], bf16, kind="ExternalInput").ap()
    out = nc.dram_tensor("out", [S, D], f32, kind="ExternalOutput").ap()

    with TileContext(nc) as tc:
        with tc.tile_pool(name="persist", bufs=1) as pp:
            # ---- persistent SBUF ----
            qt = [pp.tile([128, S], bf16, name=f"qt{i}") for i in range(2)]
            kt = [pp.tile([128, S], bf16, name=f"kt{i}") for i in range(2)]
            # fp8 V (x1.2) with rowsum column 2.0 (both exact in e4m3):
            # m1 = 1.2*PV / (2*rowsum) = 0.6*PV/rowsum exactly. Padded to 68
            # cols/head: dual-fp8 ldweights needs the outer free step 16B-
            # aligned (4*68 = 272 = 16*17) and even counts/offsets.
            v_all = pp.tile([128, NKC, NH, HD + 4], fp8)
            v_pair = pp.tile([128, NKC, CLOC], bf16)  # bf16 V for LV term
            ctxt_all = pp.tile([128, 2, S], bf16)
            wot_sb = pp.tile([128, 2, D], bf16)
            bq_sb = pp.tile([128, 2], f32)       # per-partition bias columns
            bk_sb = pp.tile([128, 2], f32)
            bv_sb = pp.tile([1, CLOC], bf16)
            lt_sb = pp.tile([128, NU, QB], bf16)
            ones_c = pp.tile([1, 128], bf16)     # ones row (V bias)

            nc.gpsimd.memset(ones_c[:], 1.0)
            nc.gpsimd.memset(v_all[:, :, :, HD:HD + 1], 2.0)
            nc.gpsimd.memset(v_all[:, :, :, HD + 1:HD + 4], 0.0)

            nc.gpsimd.dma_start(bq_sb[:], bqc[:])
            nc.gpsimd.dma_start(bk_sb[:], bkc[:])
            nc.gpsimd.dma_start(bv_sb[:], bvr[:])

            # ---- phase B: projections + LV ----
            with (
                tc.tile_pool(name="projsb", bufs=1) as prs,
                tc.tile_pool(name="projps", bufs=4, space="PSUM") as prp,
                tc.tile_pool(name="vps", bufs=4, space="PSUM") as vpp,
            ):
                xt_sb = prs.tile([128, NDC, S], bf16)
                xt8_sb = prs.tile([128, NDC, S], fp8)
                wq_sb = prs.tile([128, NDC, CLOC], fp8)
                wk_sb = prs.tile([128, NDC, CLOC], fp8)
                wv_sb = prs.tile([128, NDC, CLOC], bf16)
                # DMA issue occupies the issuing engine ~0.6ns/descriptor+
                # ~5ns/KB, so spread the 8MB of loads across the three
                # DMA-capable engines, in consumption order per engine.
                nc.scalar.dma_start(wq_sb[:, :, :],
                                    wq8.rearrange("(dc p) c -> p dc c", p=128))
                nc.scalar.dma_start(wk_sb[:, :, :],
                                    wk8.rearrange("(dc p) c -> p dc c", p=128))
                for dc in range(0, NDC, 2):  # QK inputs first, dc-granular
                    nc.scalar.dma_start(xt8_sb[:, dc, :],
                                        xt8[dc * 128:(dc + 1) * 128, :])
                    nc.sync.dma_start(xt8_sb[:, dc + 1, :],
                                      xt8[(dc + 1) * 128:(dc + 2) * 128, :])
                for dc in range(0, NDC, 2):
                    nc.sync.dma_start(xt_sb[:, dc, :],
                                      xt[dc * 128:(dc + 1) * 128, :])
                for dc in range(1, NDC, 2):
                    nc.gpsimd.dma_start(xt_sb[:, dc, :],
                                        xt[dc * 128:(dc + 1) * 128, :])
                for dc in range(NDC):
                    nc.sync.dma_start(wv_sb[:, dc, :],
                                      wvt[dc * 128:(dc + 1) * 128, :])
                nc.gpsimd.dma_start(wot_sb[:, :, :],
                                    wot.rearrange("(cc p) o -> p cc o", p=128))
                nc.gpsimd.dma_start(lt_sb[:, :, :],
                                    ltt.rearrange("u p q -> p u q"))

                # QT/KT: [c-tile 128, s-block 512], fp8 DoubleRow matmuls
                # contract 2 d-chunks per instruction at 2x rate. Weights are
                # pre-scaled x16 on host (fp8e4m3 normal range); the combined
                # 1/(16*16*8) lands in the exp's scale. Bias (x16) is added
                # on the PSUM->SBUF copy (ScalarE, per-partition bias).
                for ct in range(2):
                    for dst, w_sb, b_sb in ((qt[ct], wq_sb, bq_sb),
                                            (kt[ct], wk_sb, bk_sb)):
                        pss = [prp.tile([128, QB], f32, tag="projps",
                                        name=f"pjps{qb}") for qb in range(NQB)]
                        for dcp in range(NDC // 2):
                            for qb in range(NQB):
                                nc.tensor.matmul(
                                    pss[qb][:],
                                    w_sb[:, 2 * dcp:2 * dcp + 2,
                                         ct * 128:(ct + 1) * 128],
                                    xt8_sb[:, 2 * dcp:2 * dcp + 2,
                                           qb * QB:(qb + 1) * QB],
                                    start=(dcp == 0), stop=(dcp == NDC // 2 - 1),
                                    perf_mode=DR)
                        for qb in range(NQB):
                            nc.scalar.activation(
                                dst[:, qb * QB:(qb + 1) * QB], pss[qb][:],
                                Ident, bias=b_sb[:, ct:ct + 1])

                # V natural: [s-chunk 128, 256], contraction over d
                for sc in range(NKC):
                    ps = vpp.tile([128, CLOC], f32, tag="vps")
                    for dc in range(NDC):
                        nc.tensor.matmul(
                            ps[:],
                            xt_sb[:, dc, sc * 128:(sc + 1) * 128],
                            wv_sb[:, dc, :],
                            start=(dc == 0), stop=False)
                    nc.tensor.matmul(ps[:], ones_c[:], bv_sb[:],
                                     start=False, stop=True)
                    nc.scalar.activation(
                        v_all[:, sc, :, 0:HD],
                        ps[:].rearrange("p (h e) -> p h e", h=NH),
                        mybir.ActivationFunctionType.Copy, scale=1.2)
                    nc.vector.tensor_copy(v_pair[:, sc, :], ps[:])



            # ---- phase C: attention, software-pipelined ----
            # Heads 2hp/2hp+1 live at partitions 0-63/64-127 of c-tile hp.
            # Per (qb, hp) group: scores(kc+2)/exp(kc+1) run ahead of pv(kc)
            # so ScalarE (exp, the bottleneck) streams back-to-back while PE
            # fills its spare cycles with injected out-projection matmuls.
            with (
                tc.tile_pool(name="pt", bufs=6) as ptp,
                tc.tile_pool(name="stage", bufs=4) as sp,
                tc.tile_pool(name="osb", bufs=4) as ob,
                tc.tile_pool(name="stps", bufs=2, space="PSUM") as stp,
                tc.tile_pool(name="ctxps", bufs=3, space="PSUM") as ctp,
            ):
                def emit_outproj(sc, ot, pool, sbpool):
                    ps = pool.tile([128, QB], f32, tag="ops")
                    # cc=1 first: hp1's ctxt half is blended later, so the
                    # scheduler cannot hoist this pair to a point where it
                    # would stall PE waiting on the hp0 stg DMA.
                    for cc in (1, 0):
                        nc.tensor.matmul(
                            ps[:],
                            ctxt_all[:, cc, sc * 128:(sc + 1) * 128],
                            wot_sb[:, cc, ot * QB:(ot + 1) * QB],
                            start=(cc == 1), stop=(cc == 0),
                            skip_group_check=True)
                    o_sb = sbpool.tile([128, QB], f32, tag="osb")
                    nc.vector.tensor_copy(o_sb[:], ps[:])
                    nc.sync.dma_start(
                        out[sc * 128:(sc + 1) * 128, ot * QB:(ot + 1) * QB],
                        o_sb[:])

                def lv_closures(lvp):
                    # banded 0.4*L^T @ V, both heads per matmul (M=128),
                    # chopped into per-matmul closures injected into the
                    # first two groups' kc loops as PE filler.
                    items = []
                    for qb, hp in groups:
                        qsl = slice(qb * QB, (qb + 1) * QB)
                        slots = _LT_SLOTS[qb]
                        cell = {}

                        def mk(n, j, u, qb, hp, qsl, cell, last):
                            def go():
                                if n == 0:
                                    cell['ps'] = lvp.tile(
                                        [128, QB], f32, tag="lvps",
                                        name="lv_ps")
                                nc.tensor.matmul(
                                    cell['ps'][:],
                                    v_pair[:, j, hp * 128:(hp + 1) * 128],
                                    lt_sb[:, u, :],
                                    start=(n == 0), stop=last,
                                    skip_group_check=True)
                                if last:
                                    nc.vector.tensor_copy(
                                        ctxt_all[:, hp, qsl], cell['ps'][:])
                            return go
                        for n, (j, u) in enumerate(slots):
                            items.append(mk(n, j, u, qb, hp, qsl, cell,
                                            n == len(slots) - 1))
                    return items

                filler = []

                def mk_blend(qb, hp, css):
                    # blend: ctxt (pre-loaded with 0.4LV) += PV * 0.6/r,
                    # running entirely off the SBUF cs copies. Deferred into
                    # the NEXT group's kc loop so nothing gates a boundary.
                    # ScalarE is untouched: the partition 64->0 move is a
                    # tiny SBUF-SBUF DMA; the adds are DMA accumulates.
                    qsl = slice(qb * QB, (qb + 1) * QB)

                    def blend():
                        for hh in range(2):
                            cs = css[hh]
                            bcs = sp.tile([1, QB], f32, tag="bcs")
                            nc.sync.dma_start(bcs[0:1, :], cs[64:65, :])
                            rc = sp.tile([1, QB], f32, tag="rc")
                            nc.vector.reciprocal_approx_fast(rc[:], bcs[:])
                            bc = sp.tile([64, QB], f32, tag="bc")
                            nc.gpsimd.partition_broadcast(
                                bc[:], rc[0:1, :], channels=HD)
                            m1 = sp.tile([64, QB], bf16, tag="m1")
                            nc.vector.tensor_mul(m1[:], cs[0:HD, :], bc[:])
                            nc.gpsimd.dma_start(
                                ctxt_all[hh * 64:hh * 64 + 64, hp, qsl],
                                m1[:], accum_op=add)
                    return blend

                def emit_group(qb, hp, prev_blend, pops):
                    qsl = slice(qb * QB, (qb + 1) * QB)
                    ctx = [ctp.tile([128, QB], f32, tag="ctxps",
                                    name=f"ctx{hh}") for hh in range(2)]
                    sts, pts = {}, {}

                    def scores(kc):
                        st = stp.tile([128, 2, QB], f32, tag="stps")
                        sts[kc] = st
                        for hh in range(2):
                            p0 = hh * 64
                            nc.tensor.matmul(
                                st[:, hh, :],
                                kt[hp][p0:p0 + 64, kc * 128:(kc + 1) * 128],
                                qt[hp][p0:p0 + 64, qsl],
                                start=True, stop=True,
                                tile_position=(p0, 0))
                        if kc % 2 == 0:
                            pts[kc // 2] = ptp.tile([128, 2, 2, QB], fp8,
                                                    tag="pt", name="ptt")
                        nc.scalar.activation(pts[kc // 2][:, kc % 2, :, :],
                                             st[:], Exp, scale=1.0 / 2048.0)

                    def pv_pair(kcp):
                        # fp8 DoubleRow: contract 2 k-chunks per instruction
                        ptt = pts.pop(kcp)
                        sts.pop(2 * kcp, None)
                        sts.pop(2 * kcp + 1, None)
                        for hh in range(2):
                            nc.tensor.matmul(
                                ctx[hh][0:HD + 4, :],
                                v_all[:, 2 * kcp:2 * kcp + 2,
                                      2 * hp + hh, 0:HD + 4],
                                ptt[:, :, hh, :],
                                start=(kcp == 0), stop=(kcp == NKC // 2 - 1),
                                perf_mode=DR)

                    scores(0)
                    scores(1)
                    for kc in range(NKC):
                        if kc + 2 < NKC:
                            scores(kc + 2)
                        if kc % 2 == 1:
                            pv_pair(kc // 2)
                        if kc == 1 and prev_blend is not None:
                            prev_blend()
                        for _ in range(pops(kc)):
                            if filler:
                                filler.pop(0)()
                    # copy ctx off PSUM right away: the banks free ~1us after
                    # pv(15), so the next group's accumulation never stalls
                    css = []
                    for hh in range(2):
                        cs = sp.tile([65, QB], f32, tag="cs")
                        nc.vector.tensor_copy(cs[:], ctx[hh][0:HD + 1, :])
                        css.append(cs)
                    return mk_blend(qb, hp, css)

                # (qb, hp) order chosen so out-proj(qb) becomes injectable
                # 1-2 groups after both hp halves of qb have blended.
                groups = [(0, 0), (0, 1), (1, 0), (1, 1),
                          (2, 0), (2, 1), (3, 0), (3, 1)]
                lv_pops = lambda kc: 2
                op_pops = lambda kc: 1 if kc in (3, 5, 8, 11, 14) else 0
                push_after = {2: 0, 4: 1, 6: 2}
                pending_blend = None
                # g0/g1: LV matmuls (PE filler) under a 1-bank PSUM pool that
                # closes before the out-projection pool opens.
                with tc.tile_pool(name="lvps", bufs=1, space="PSUM") as lvp:
                    filler.extend(lv_closures(lvp))
                    for gi in (0, 1):
                        qb, hp = groups[gi]
                        pending_blend = emit_group(qb, hp, pending_blend,
                                                   lv_pops)
                    while filler:  # safety: drain any LV leftovers
                        filler.pop(0)()
                opq = [(sc, ot) for qb_o in range(3)
                       for ot in range(2) for sc in range(qb_o * 4, qb_o * 4 + 4)]
                op_state = {"pushed": 0, "popped": 0}
                with tc.tile_pool(name="ops", bufs=1, space="PSUM") as opp:
                    def op_closure():
                        if op_state["popped"] < op_state["pushed"]:
                            sc, ot = opq[op_state["popped"]]
                            op_state["popped"] += 1
                            emit_outproj(sc, ot, opp, ob)
                    filler.extend([op_closure] * 64)  # pops gated by pushed
                    for gi in range(2, 8):
                        qb, hp = groups[gi]
                        pending_blend = emit_group(qb, hp, pending_blend,
                                                   op_pops)
                        if gi in push_after:
                            op_state["pushed"] = 8 * (push_after[gi] + 1)
                    pending_blend()
                    leftovers = opq[op_state["popped"]:]
                    filler.clear()

            # ---- phase D: out-projection tail (wider PSUM pool) ----
            with (
                tc.tile_pool(name="ops2", bufs=4, space="PSUM") as opp2,
                tc.tile_pool(name="osb2", bufs=4) as ob2,
            ):
                for sc, ot in leftovers:
                    emit_outproj(sc, ot, opp2, ob2)
                for sc in range(12, 16):
                    for ot in range(2):
                        emit_outproj(sc, ot, opp2, ob2)

    nc.compile()
    return nc


def _get_program():
    if "nc" not in _CACHE:
        _CACHE["nc"] = _build_program()
    return _CACHE["nc"]


def _in_maps(x, Wq, bq, Wk, bk, Wv, bv, Wo):
    FP8 = ml_dtypes.float8_e4m3fn
    xT = [np.ascontiguousarray(x[b].T).astype(BF16) for b in range(2)]
    xT8 = [np.ascontiguousarray(x[b].T).astype(FP8) for b in range(2)]
    maps = []
    for c in range(8):
        b, hg = c // 4, c % 4
        hs, he = hg * CLOC, (hg + 1) * CLOC
        maps.append({
            "xt": xT[b],
            "xt8": xT8[b],
            "wq8": np.ascontiguousarray(Wq[hs:he].T * F32(16.0)).astype(FP8),
            "wk8": np.ascontiguousarray(Wk[hs:he].T * F32(16.0)).astype(FP8),
            "wvt": np.ascontiguousarray(Wv[hs:he].T).astype(BF16),
            "bqc": np.ascontiguousarray((bq[hs:he] * F32(16.0)).reshape(2, 128).T),
            "bkc": np.ascontiguousarray((bk[hs:he] * F32(16.0)).reshape(2, 128).T),
            "bvr": bv[hs:he][None, :].astype(BF16),
            "wot": np.ascontiguousarray(Wo[:, hs:he].T).astype(BF16),
            "ltt": _LT_UNIQ,
        })
    return maps


def _run(x, Wq, bq, Wk, bk, Wv, bv, Wo, bo, trace=False):
    from concourse.bass_utils import run_bass_kernel_spmd
    nc = _get_program()
    maps = _in_maps(np.asarray(x, F32), np.asarray(Wq, F32), np.asarray(bq, F32),
                    np.asarray(Wk, F32), np.asarray(bk, F32), np.asarray(Wv, F32),
                    np.asarray(bv, F32), np.asarray(Wo, F32))
    res = run_bass_kernel_spmd(nc, maps, list(range(8)), trace=trace)
    bo = np.asarray(bo, F32)
    outp = np.empty((2, S, D), F32)
    for b in range(2):
        acc = res.results[b * 4]["out"].astype(F32)
        for hg in range(1, 4):
            acc = acc + res.results[b * 4 + hg]["out"]
        outp[b] = acc + bo
    return outp, res


def kernel(x, Wq, bq, Wk, bk, Wv, bv, Wo, bo):
    outp, _ = _run(x, Wq, bq, Wk, bk, Wv, bv, Wo, bo, trace=False)
    return outp


def kernel_traced(**inputs):
    return _run(trace=True, **inputs)


# revision 52
# speedup vs baseline: 1.0910x; 1.0320x over previous
"""Trainium2 Bass kernel for DampedAttention.

Full inputs in, full output out. Sharding: 8 cores = 2 batches x 4 head-groups
(4 heads of dim 64 each per core). Per core:

  QT/KT  [c, s] transposed projections (c on partitions), scale 1/8 folded
         into wq/bq on host; bias folded via per-partition activation bias
         on the PSUM->SBUF copy (ScalarE, idle during projections)
  V      [s, c] natural projection (lhsT for the P@V matmul); bias via a
         K=1 ones-row matmul
  LV     banded 0.4*L^T term precomputed per (hp, qb) into SBUF during the
         projection phase; both heads per matmul (lhsT = [V_h0|V_h1], M=128)
  ST     scores transposed [k, q] per (k-chunk, q-block) so exp(ST) is the
         lhsT-layout P^T needed by P@V -- no on-chip transposes
  exp    software-pipelined: scores(kc+2) and exp(kc+1) run ahead of pv(kc)
         so ScalarE (the bottleneck) streams exps back-to-back
  ctxT   [65, q] = V_aug^T @ P^T ; row 64 = softmax row-sums (ones column)
  blend  ctxT_final = PV * (0.6/r) + 0.4LV; 1/r via DVE reciprocal_approx
         + gpsimd partition broadcast (keeps ScalarE exp-only: one act table)
  out    [s, o] out-projection matmuls injected into later attention groups'
         loops as PE filler; host sums 4 head-group partials + bo

Matmul operands are bf16; accumulation, row-sums, reciprocal and the 0.6/r
normalization stay fp32. The entropy gate in the reference is a forward
no-op and is skipped. Softmax max-subtraction is skipped (scores are O(1)).
"""
import numpy as np
import ml_dtypes

S = 2048
D = 1024
CLOC = 256          # channels per core (4 heads x 64)
HD = 64
NH = 4              # heads per core
NDC = 8             # 128-wide d-chunks in contraction D
NKC = 16            # 128-wide k/s chunks in S
NQB = 4             # 512-wide q blocks
QB = 512
WINDOW = 3
STRENGTH = 0.4
EPS = 1e-10
F32 = np.float32
BF16 = ml_dtypes.bfloat16


def _build_L04T():
    i = np.arange(S)
    d = (i[:, None] - i[None, :]).astype(F32)
    k = np.where(np.abs(d) <= WINDOW,
                 np.exp(-(d ** 2) / F32(2.0 * STRENGTH ** 2)),
                 F32(0.0)).astype(F32)
    L = k / (k.sum(axis=-1, keepdims=True) + F32(EPS))
    return (F32(0.4) * L).T.copy()  # [s, q], pre-scaled by (1 - lambda_jump)


def _lt_tiles():
    """Unique [128, 512] band tiles of 0.4*L^T plus (qb -> [(j, uniq_idx)])."""
    L04T = _build_L04T()
    uniq = []
    slots = {qb: [] for qb in range(NQB)}
    for qb in range(NQB):
        for j in range(max(0, qb * 4 - 1), min(NKC, qb * 4 + 5)):
            t = L04T[j * 128:(j + 1) * 128, qb * QB:(qb + 1) * QB]
            for ui, ut in enumerate(uniq):
                if np.array_equal(t, ut):
                    slots[qb].append((j, ui))
                    break
            else:
                slots[qb].append((j, len(uniq)))
                uniq.append(t)
    return np.stack(uniq).astype(BF16), slots


_LT_UNIQ, _LT_SLOTS = _lt_tiles()
NU = _LT_UNIQ.shape[0]

_CACHE = {}


def _build_program():
    import concourse.bacc as bacc
    import concourse.mybir as mybir
    from concourse.tile import TileContext

    f32 = mybir.dt.float32
    bf16 = mybir.dt.bfloat16
    fp8 = mybir.dt.float8e4
    Exp = mybir.ActivationFunctionType.Exp
    Ident = mybir.ActivationFunctionType.Identity
    mult = mybir.AluOpType.mult
    add = mybir.AluOpType.add
    DR = mybir.MatmulPerfMode.DoubleRow

    nc = bacc.Bacc("TRN2", target_bir_lowering=False, debug=False,
                   enable_asserts=False, num_devices=8)

    xt = nc.dram_tensor("xt", [D, S], bf16, kind="ExternalInput").ap()
    xt8 = nc.dram_tensor("xt8", [D, S], fp8, kind="ExternalInput").ap()
    wq8 = nc.dram_tensor("wq8", [D, CLOC], fp8, kind="ExternalInput").ap()
    wk8 = nc.dram_tensor("wk8", [D, CLOC], fp8, kind="ExternalInput").ap()
    wvt = nc.dram_tensor("wvt", [D, CLOC], bf16, kind="ExternalInput").ap()
    bqc = nc.dram_tensor("bqc", [128, 2], f32, kind="ExternalInput").ap()
    bkc = nc.dram_tensor("bkc", [128, 2], f32, kind="ExternalInput").ap()
    bvr = nc.dram_tensor("bvr", [1, CLOC], bf16, kind="ExternalInput").ap()
    wot = nc.dram_tensor("wot", [CLOC, D], bf16, kind="ExternalInput").ap()
    ltt = nc.dram_tensor("ltt", [NU, 128, QB], bf16, kind="ExternalInput").ap()
    out = nc.dram_tensor("out", [S, D], f32, kind="ExternalOutput").ap()

    with TileContext(nc) as tc:
        with tc.tile_pool(name="persist", bufs=1) as pp:
            # ---- persistent SBUF ----
            qt = [pp.tile([128, S], bf16, name=f"qt{i}") for i in range(2)]
            kt = [pp.tile([128, S], bf16, name=f"kt{i}") for i in range(2)]
            # fp8 V (x1.2) with rowsum column 2.0 (both exact in e4m3):
            # m1 = 1.2*PV / (2*rowsum) = 0.6*PV/rowsum exactly. Padded to 68
            # cols/head: dual-fp8 ldweights needs the outer free step 16B-
            # aligned (4*68 = 272 = 16*17) and even counts/offsets.
            v_all = pp.tile([128, NKC, NH, HD + 4], fp8)
            v_pair = pp.tile([128, NKC, CLOC], bf16)  # bf16 V for LV term
            ctxt_q = [pp.tile([128, 2, QB], bf16, name=f"ctxt{qb}")
                      for qb in range(NQB)]
            wot_sb = pp.tile([128, 2, D], bf16)
            bq_sb = pp.tile([128, 2], f32)       # per-partition bias columns
            bk_sb = pp.tile([128, 2], f32)
            bv_sb = pp.tile([1, CLOC], bf16)
            lt_sb = pp.tile([128, NU, QB], bf16)
            ones_c = pp.tile([1, 128], bf16)     # ones row (V bias)

            nc.gpsimd.memset(ones_c[:], 1.0)
            nc.gpsimd.memset(v_all[:, :, :, HD:HD + 1], 2.0)
            nc.gpsimd.memset(v_all[:, :, :, HD + 1:HD + 4], 0.0)

            nc.gpsimd.dma_start(bq_sb[:], bqc[:])
            nc.gpsimd.dma_start(bk_sb[:], bkc[:])
            nc.gpsimd.dma_start(bv_sb[:], bvr[:])

            # ---- phase B: projections + LV ----
            with (
                tc.tile_pool(name="projsb", bufs=1) as prs,
                tc.tile_pool(name="projps", bufs=4, space="PSUM") as prp,
                tc.tile_pool(name="vps", bufs=4, space="PSUM") as vpp,
            ):
                xt_sb = prs.tile([128, NDC, S], bf16)
                xt8_sb = prs.tile([128, NDC, S], fp8)
                wq_sb = prs.tile([128, NDC, CLOC], fp8)
                wk_sb = prs.tile([128, NDC, CLOC], fp8)
                wv_sb = prs.tile([128, NDC, CLOC], bf16)
                # DMA issue occupies the issuing engine ~0.6ns/descriptor+
                # ~5ns/KB, so spread the 8MB of loads across the three
                # DMA-capable engines, in consumption order per engine.
                for dc in range(0, NDC, 2):  # QK inputs first, dc-granular
                    nc.scalar.dma_start(
                        wq_sb[:, dc:dc + 2, :],
                        wq8[dc * 128:(dc + 2) * 128, :].rearrange(
                            "(two p) c -> p two c", p=128))
                    nc.scalar.dma_start(xt8_sb[:, dc, :],
                                        xt8[dc * 128:(dc + 1) * 128, :])
                    nc.sync.dma_start(xt8_sb[:, dc + 1, :],
                                      xt8[(dc + 1) * 128:(dc + 2) * 128, :])
                    nc.sync.dma_start(
                        wk_sb[:, dc:dc + 2, :],
                        wk8[dc * 128:(dc + 2) * 128, :].rearrange(
                            "(two p) c -> p two c", p=128))
                for dc in range(0, NDC, 2):
                    nc.sync.dma_start(xt_sb[:, dc, :],
                                      xt[dc * 128:(dc + 1) * 128, :])
                for dc in range(1, NDC, 2):
                    nc.gpsimd.dma_start(xt_sb[:, dc, :],
                                        xt[dc * 128:(dc + 1) * 128, :])
                for dc in range(NDC):
                    nc.sync.dma_start(wv_sb[:, dc, :],
                                      wvt[dc * 128:(dc + 1) * 128, :])
                nc.gpsimd.dma_start(wot_sb[:, :, :],
                                    wot.rearrange("(cc p) o -> p cc o", p=128))
                nc.gpsimd.dma_start(lt_sb[:, :, :],
                                    ltt.rearrange("u p q -> p u q"))

                # QT/KT: [c-tile 128, s-block 512], fp8 DoubleRow matmuls
                # contract 2 d-chunks per instruction at 2x rate. Weights are
                # pre-scaled x16 on host (fp8e4m3 normal range); the combined
                # 1/(16*16*8) lands in the exp's scale. Bias (x16) is added
                # on the PSUM->SBUF copy (ScalarE, per-partition bias).
                for ct in range(2):
                    for dst, w_sb, b_sb in ((qt[ct], wq_sb, bq_sb),
                                            (kt[ct], wk_sb, bk_sb)):
                        pss = [prp.tile([128, QB], f32, tag="projps",
                                        name=f"pjps{qb}") for qb in range(NQB)]
                        for dcp in range(NDC // 2):
                            for qb in range(NQB):
                                nc.tensor.matmul(
                                    pss[qb][:],
                                    w_sb[:, 2 * dcp:2 * dcp + 2,
                                         ct * 128:(ct + 1) * 128],
                                    xt8_sb[:, 2 * dcp:2 * dcp + 2,
                                           qb * QB:(qb + 1) * QB],
                                    start=(dcp == 0), stop=(dcp == NDC // 2 - 1),
                                    perf_mode=DR)
                        for qb in range(NQB):
                            nc.scalar.activation(
                                dst[:, qb * QB:(qb + 1) * QB], pss[qb][:],
                                Ident, bias=b_sb[:, ct:ct + 1])

                # V natural: [s-chunk 128, 256], contraction over d
                for sc in range(NKC):
                    ps = vpp.tile([128, CLOC], f32, tag="vps")
                    for dc in range(NDC):
                        nc.tensor.matmul(
                            ps[:],
                            xt_sb[:, dc, sc * 128:(sc + 1) * 128],
                            wv_sb[:, dc, :],
                            start=(dc == 0), stop=False)
                    nc.tensor.matmul(ps[:], ones_c[:], bv_sb[:],
                                     start=False, stop=True)
                    nc.scalar.activation(
                        v_all[:, sc, :, 0:HD],
                        ps[:].rearrange("p (h e) -> p h e", h=NH),
                        mybir.ActivationFunctionType.Copy, scale=1.2)
                    nc.vector.tensor_copy(v_pair[:, sc, :], ps[:])



            # ---- phase C: attention, software-pipelined ----
            # Heads 2hp/2hp+1 live at partitions 0-63/64-127 of c-tile hp.
            # Per (qb, hp) group: scores(kc+2)/exp(kc+1) run ahead of pv(kc)
            # so ScalarE (exp, the bottleneck) streams back-to-back while PE
            # fills its spare cycles with injected out-projection matmuls.
            with (
                tc.tile_pool(name="pt", bufs=6) as ptp,
                tc.tile_pool(name="stage", bufs=4) as sp,
                tc.tile_pool(name="osb", bufs=4) as ob,
                tc.tile_pool(name="stps", bufs=2, space="PSUM") as stp,
                tc.tile_pool(name="ctxps", bufs=3, space="PSUM") as ctp,
            ):
                def emit_outproj(sc, ot, pool, sbpool):
                    ps = pool.tile([128, QB], f32, tag="ops")
                    # cc=1 first: hp1's ctxt half is blended later, so the
                    # scheduler cannot hoist this pair to a point where it
                    # would stall PE waiting on the hp0 stg DMA.
                    for cc in (1, 0):
                        nc.tensor.matmul(
                            ps[:],
                            ctxt_q[sc // 4][:, cc,
                                            (sc % 4) * 128:(sc % 4 + 1) * 128],
                            wot_sb[:, cc, ot * QB:(ot + 1) * QB],
                            start=(cc == 1), stop=(cc == 0),
                            skip_group_check=True)
                    o_sb = sbpool.tile([128, QB], f32, tag="osb")
                    nc.vector.tensor_copy(o_sb[:], ps[:])
                    nc.sync.dma_start(
                        out[sc * 128:(sc + 1) * 128, ot * QB:(ot + 1) * QB],
                        o_sb[:])

                def lv_closures(lvp):
                    # banded 0.4*L^T @ V, both heads per matmul (M=128),
                    # chopped into per-matmul closures injected into the
                    # first two groups' kc loops as PE filler.
                    items = []
                    for qb, hp in groups:
                        slots = _LT_SLOTS[qb]
                        cell = {}

                        def mk(n, j, u, qb, hp, cell, last):
                            def go():
                                if n == 0:
                                    cell['ps'] = lvp.tile(
                                        [128, QB], f32, tag="lvps",
                                        name="lv_ps")
                                nc.tensor.matmul(
                                    cell['ps'][:],
                                    v_pair[:, j, hp * 128:(hp + 1) * 128],
                                    lt_sb[:, u, :],
                                    start=(n == 0), stop=last,
                                    skip_group_check=True)
                                if last:
                                    nc.vector.tensor_copy(
                                        ctxt_q[qb][:, hp, :], cell['ps'][:])
                            return go
                        for n, (j, u) in enumerate(slots):
                            items.append(mk(n, j, u, qb, hp, cell,
                                            n == len(slots) - 1))
                    return items

                filler = []

                def mk_blend(qb, hp, css):
                    # blend: ctxt (pre-loaded with 0.4LV) += PV * 0.6/r,
                    # running entirely off the SBUF cs copies. Deferred into
                    # the NEXT group's kc loop so nothing gates a boundary.
                    # ScalarE is untouched: the partition 64->0 move is a
                    # tiny SBUF-SBUF DMA; the adds are DMA accumulates.
                    qsl = slice(qb * QB, (qb + 1) * QB)

                    def blend():
                        for hh in range(2):
                            cs = css[hh]
                            bcs = sp.tile([1, QB], f32, tag="bcs")
                            nc.sync.dma_start(bcs[0:1, :], cs[64:65, :])
                            rc = sp.tile([1, QB], f32, tag="rc")
                            nc.vector.reciprocal_approx_fast(rc[:], bcs[:])
                            bc = sp.tile([64, QB], f32, tag="bc")
                            nc.gpsimd.partition_broadcast(
                                bc[:], rc[0:1, :], channels=HD)
                            m1 = sp.tile([64, QB], bf16, tag="m1")
                            nc.vector.tensor_mul(m1[:], cs[0:HD, :], bc[:])
                            if hh == 0:
                                nc.vector.tensor_add(
                                    ctxt_q[qb][0:64, hp, :], m1[:],
                                    ctxt_q[qb][0:64, hp, :])
                            else:
                                nc.gpsimd.dma_start(
                                    ctxt_q[qb][64:128, hp, :],
                                    m1[:], accum_op=add)
                    return blend

                def emit_group(qb, hp, prev_blend, pops):
                    qsl = slice(qb * QB, (qb + 1) * QB)
                    ctx = [ctp.tile([128, QB], f32, tag="ctxps",
                                    name=f"ctx{hh}") for hh in range(2)]
                    sts, pts = {}, {}

                    def scores(kc):
                        st = stp.tile([128, 2, QB], f32, tag="stps")
                        sts[kc] = st
                        for hh in range(2):
                            p0 = hh * 64
                            nc.tensor.matmul(
                                st[:, hh, :],
                                kt[hp][p0:p0 + 64, kc * 128:(kc + 1) * 128],
                                qt[hp][p0:p0 + 64, qsl],
                                start=True, stop=True,
                                tile_position=(p0, 0))
                        if kc % 2 == 0:
                            pts[kc // 2] = ptp.tile([128, 2, 2, QB], fp8,
                                                    tag="pt", name="ptt")
                        nc.scalar.activation(pts[kc // 2][:, kc % 2, :, :],
                                             st[:], Exp, scale=1.0 / 2048.0)

                    def pv_pair(kcp):
                        # fp8 DoubleRow: contract 2 k-chunks per instruction
                        ptt = pts.pop(kcp)
                        sts.pop(2 * kcp, None)
                        sts.pop(2 * kcp + 1, None)
                        for hh in range(2):
                            nc.tensor.matmul(
                                ctx[hh][0:HD + 4, :],
                                v_all[:, 2 * kcp:2 * kcp + 2,
                                      2 * hp + hh, 0:HD + 4],
                                ptt[:, :, hh, :],
                                start=(kcp == 0), stop=(kcp == NKC // 2 - 1),
                                perf_mode=DR)

                    scores(0)
                    scores(1)
                    for kc in range(NKC):
                        if kc + 2 < NKC:
                            scores(kc + 2)
                        if kc % 2 == 1:
                            pv_pair(kc // 2)
                        if kc == 1 and prev_blend is not None:
                            prev_blend()
                        for _ in range(pops(kc)):
                            if filler:
                                filler.pop(0)()
                    # copy ctx off PSUM right away: the banks free ~1us after
                    # pv(15), so the next group's accumulation never stalls
                    css = []
                    for hh in range(2):
                        cs = sp.tile([65, QB], f32, tag="cs")
                        nc.vector.tensor_copy(cs[:], ctx[hh][0:HD + 1, :])
                        css.append(cs)
                    return mk_blend(qb, hp, css)

                # (qb, hp) order chosen so out-proj(qb) becomes injectable
                # 1-2 groups after both hp halves of qb have blended.
                groups = [(0, 0), (0, 1), (1, 0), (1, 1),
                          (2, 0), (2, 1), (3, 0), (3, 1)]
                lv_pops = lambda kc: 2
                op_pops = lambda kc: 1 if kc in (3, 5, 8, 11, 14) else 0
                push_after = {2: 0, 4: 1, 6: 2}
                pending_blend = None
                # g0/g1: LV matmuls (PE filler) under a 1-bank PSUM pool that
                # closes before the out-projection pool opens.
                with tc.tile_pool(name="lvps", bufs=1, space="PSUM") as lvp:
                    filler.extend(lv_closures(lvp))
                    for gi in (0, 1):
                        qb, hp = groups[gi]
                        pending_blend = emit_group(qb, hp, pending_blend,
                                                   lv_pops)
                    while filler:  # safety: drain any LV leftovers
                        filler.pop(0)()
                opq = [(sc, ot) for qb_o in range(3)
                       for ot in range(2) for sc in range(qb_o * 4, qb_o * 4 + 4)]
                op_state = {"pushed": 0, "popped": 0}
                with tc.tile_pool(name="ops", bufs=1, space="PSUM") as opp:
                    def op_closure():
                        if op_state["popped"] < op_state["pushed"]:
                            sc, ot = opq[op_state["popped"]]
                            op_state["popped"] += 1
                            emit_outproj(sc, ot, opp, ob)
                    filler.extend([op_closure] * 64)  # pops gated by pushed
                    for gi in range(2, 8):
                        qb, hp = groups[gi]
                        pending_blend = emit_group(qb, hp, pending_blend,
                                                   op_pops)
                        if gi in push_after:
                            op_state["pushed"] = 8 * (push_after[gi] + 1)
                    pending_blend()
                    leftovers = opq[op_state["popped"]:]
                    filler.clear()

            # ---- phase D: out-projection tail (wider PSUM pool) ----
            with (
                tc.tile_pool(name="ops2", bufs=4, space="PSUM") as opp2,
                tc.tile_pool(name="osb2", bufs=4) as ob2,
            ):
                for sc, ot in leftovers:
                    emit_outproj(sc, ot, opp2, ob2)
                for sc in range(12, 16):
                    for ot in range(2):
                        emit_outproj(sc, ot, opp2, ob2)

    nc.compile()
    return nc


def _get_program():
    if "nc" not in _CACHE:
        _CACHE["nc"] = _build_program()
    return _CACHE["nc"]


def _in_maps(x, Wq, bq, Wk, bk, Wv, bv, Wo):
    FP8 = ml_dtypes.float8_e4m3fn
    xT = [np.ascontiguousarray(x[b].T).astype(BF16) for b in range(2)]
    xT8 = [np.ascontiguousarray(x[b].T).astype(FP8) for b in range(2)]
    maps = []
    for c in range(8):
        b, hg = c // 4, c % 4
        hs, he = hg * CLOC, (hg + 1) * CLOC
        maps.append({
            "xt": xT[b],
            "xt8": xT8[b],
            "wq8": np.ascontiguousarray(Wq[hs:he].T * F32(16.0)).astype(FP8),
            "wk8": np.ascontiguousarray(Wk[hs:he].T * F32(16.0)).astype(FP8),
            "wvt": np.ascontiguousarray(Wv[hs:he].T).astype(BF16),
            "bqc": np.ascontiguousarray((bq[hs:he] * F32(16.0)).reshape(2, 128).T),
            "bkc": np.ascontiguousarray((bk[hs:he] * F32(16.0)).reshape(2, 128).T),
            "bvr": bv[hs:he][None, :].astype(BF16),
            "wot": np.ascontiguousarray(Wo[:, hs:he].T).astype(BF16),
            "ltt": _LT_UNIQ,
        })
    return maps


def _run(x, Wq, bq, Wk, bk, Wv, bv, Wo, bo, trace=False):
    from concourse.bass_utils import run_bass_kernel_spmd
    nc = _get_program()
    maps = _in_maps(np.asarray(x, F32), np.asarray(Wq, F32), np.asarray(bq, F32),
                    np.asarray(Wk, F32), np.asarray(bk, F32), np.asarray(Wv, F32),
                    np.asarray(bv, F32), np.asarray(Wo, F32))
    res = run_bass_kernel_spmd(nc, maps, list(range(8)), trace=trace)
    bo = np.asarray(bo, F32)
    outp = np.empty((2, S, D), F32)
    for b in range(2):
        acc = res.results[b * 4]["out"].astype(F32)
        for hg in range(1, 4):
            acc = acc + res.results[b * 4 + hg]["out"]
        outp[b] = acc + bo
    return outp, res


def kernel(x, Wq, bq, Wk, bk, Wv, bv, Wo, bo):
    outp, _ = _run(x, Wq, bq, Wk, bk, Wv, bv, Wo, bo, trace=False)
    return outp


def kernel_traced(**inputs):
    return _run(trace=True, **inputs)


# revision 54
# speedup vs baseline: 1.1521x; 1.0560x over previous
"""Trainium2 Bass kernel for DampedAttention.

Full inputs in, full output out. Sharding: 8 cores = 2 batches x 4 head-groups
(4 heads of dim 64 each per core). Per core:

  QT/KT  [c, s] transposed projections (c on partitions), scale 1/8 folded
         into wq/bq on host; bias folded via per-partition activation bias
         on the PSUM->SBUF copy (ScalarE, idle during projections)
  V      [s, c] natural projection (lhsT for the P@V matmul); bias via a
         K=1 ones-row matmul
  LV     banded 0.4*L^T term precomputed per (hp, qb) into SBUF during the
         projection phase; both heads per matmul (lhsT = [V_h0|V_h1], M=128)
  ST     scores transposed [k, q] per (k-chunk, q-block) so exp(ST) is the
         lhsT-layout P^T needed by P@V -- no on-chip transposes
  exp    software-pipelined: scores(kc+2) and exp(kc+1) run ahead of pv(kc)
         so ScalarE (the bottleneck) streams exps back-to-back
  ctxT   [65, q] = V_aug^T @ P^T ; row 64 = softmax row-sums (ones column)
  blend  ctxT_final = PV * (0.6/r) + 0.4LV; 1/r via DVE reciprocal_approx
         + gpsimd partition broadcast (keeps ScalarE exp-only: one act table)
  out    [s, o] out-projection matmuls injected into later attention groups'
         loops as PE filler; host sums 4 head-group partials + bo

Matmul operands are bf16; accumulation, row-sums, reciprocal and the 0.6/r
normalization stay fp32. The entropy gate in the reference is a forward
no-op and is skipped. Softmax max-subtraction is skipped (scores are O(1)).
"""
import numpy as np
import ml_dtypes

S = 2048
D = 1024
CLOC = 256          # channels per core (4 heads x 64)
HD = 64
NH = 4              # heads per core
NDC = 8             # 128-wide d-chunks in contraction D
NKC = 16            # 128-wide k/s chunks in S
NQB = 4             # 512-wide q blocks
QB = 512
WINDOW = 3
STRENGTH = 0.4
EPS = 1e-10
F32 = np.float32
BF16 = ml_dtypes.bfloat16


def _build_L04T():
    i = np.arange(S)
    d = (i[:, None] - i[None, :]).astype(F32)
    k = np.where(np.abs(d) <= WINDOW,
                 np.exp(-(d ** 2) / F32(2.0 * STRENGTH ** 2)),
                 F32(0.0)).astype(F32)
    L = k / (k.sum(axis=-1, keepdims=True) + F32(EPS))
    return (F32(0.4) * L).T.copy()  # [s, q], pre-scaled by (1 - lambda_jump)


def _lt_tiles():
    """Unique [128, 512] band tiles of 0.4*L^T plus (qb -> [(j, uniq_idx)])."""
    L04T = _build_L04T()
    uniq = []
    slots = {qb: [] for qb in range(NQB)}
    for qb in range(NQB):
        for j in range(max(0, qb * 4 - 1), min(NKC, qb * 4 + 5)):
            t = L04T[j * 128:(j + 1) * 128, qb * QB:(qb + 1) * QB]
            for ui, ut in enumerate(uniq):
                if np.array_equal(t, ut):
                    slots[qb].append((j, ui))
                    break
            else:
                slots[qb].append((j, len(uniq)))
                uniq.append(t)
    return np.stack(uniq).astype(BF16), slots


_LT_UNIQ, _LT_SLOTS = _lt_tiles()
NU = _LT_UNIQ.shape[0]

_CACHE = {}


def _build_program():
    import concourse.bacc as bacc
    import concourse.mybir as mybir
    from concourse.tile import TileContext

    f32 = mybir.dt.float32
    bf16 = mybir.dt.bfloat16
    fp8 = mybir.dt.float8e4
    Exp = mybir.ActivationFunctionType.Exp
    Ident = mybir.ActivationFunctionType.Identity
    mult = mybir.AluOpType.mult
    add = mybir.AluOpType.add
    DR = mybir.MatmulPerfMode.DoubleRow

    nc = bacc.Bacc("TRN2", target_bir_lowering=False, debug=False,
                   enable_asserts=False, num_devices=8)

    xt = nc.dram_tensor("xt", [D, S], bf16, kind="ExternalInput").ap()
    xt8 = nc.dram_tensor("xt8", [D, S], fp8, kind="ExternalInput").ap()
    wq8 = nc.dram_tensor("wq8", [D, CLOC], fp8, kind="ExternalInput").ap()
    wk8 = nc.dram_tensor("wk8", [D, CLOC], fp8, kind="ExternalInput").ap()
    wvt = nc.dram_tensor("wvt", [D, CLOC], bf16, kind="ExternalInput").ap()
    bqc = nc.dram_tensor("bqc", [128, 2], f32, kind="ExternalInput").ap()
    bkc = nc.dram_tensor("bkc", [128, 2], f32, kind="ExternalInput").ap()
    bvr = nc.dram_tensor("bvr", [1, CLOC], bf16, kind="ExternalInput").ap()
    wot = nc.dram_tensor("wot", [CLOC, D], bf16, kind="ExternalInput").ap()
    ltt = nc.dram_tensor("ltt", [NU, 128, QB], bf16, kind="ExternalInput").ap()
    out = nc.dram_tensor("out", [S, D], f32, kind="ExternalOutput").ap()

    with TileContext(nc) as tc:
        with tc.tile_pool(name="persist", bufs=1) as pp:
            # ---- persistent SBUF ----
            qt = [pp.tile([128, S], bf16, name=f"qt{i}") for i in range(2)]
            kt = [pp.tile([128, S], bf16, name=f"kt{i}") for i in range(2)]
            # fp8 V (x1.2) with rowsum column 2.0 (both exact in e4m3):
            # m1 = 1.2*PV / (2*rowsum) = 0.6*PV/rowsum exactly. Padded to 68
            # cols/head: dual-fp8 ldweights needs the outer free step 16B-
            # aligned (4*68 = 272 = 16*17) and even counts/offsets.
            v_all = pp.tile([128, NKC, NH, HD + 4], fp8)
            v_pair = pp.tile([128, NKC, CLOC], bf16)  # bf16 V for LV term
            ctxt_q = [pp.tile([128, 2, QB], bf16, name=f"ctxt{qb}")
                      for qb in range(NQB)]
            wot_sb = pp.tile([128, 2, D], bf16)
            bq_sb = pp.tile([128, 2], f32)       # per-partition bias columns
            bk_sb = pp.tile([128, 2], f32)
            bv_sb = pp.tile([1, CLOC], bf16)
            lt_sb = pp.tile([128, NU, QB], bf16)
            ones_c = pp.tile([1, 128], bf16)     # ones row (V bias)

            nc.gpsimd.memset(ones_c[:], 1.0)
            nc.gpsimd.memset(v_all[:, :, :, HD:HD + 1], 2.0)
            nc.gpsimd.memset(v_all[:, :, :, HD + 1:HD + 4], 0.0)
            # dummy broadcast: loads the gpsimd broadcast ucode library now
            # (~7us) instead of stalling the first blend chain mid-attention
            warm = pp.tile([64, 32], bf16, name="warm")
            nc.gpsimd.partition_broadcast(warm[:], ones_c[0:1, 0:32],
                                          channels=HD)

            nc.gpsimd.dma_start(bq_sb[:], bqc[:])
            nc.gpsimd.dma_start(bk_sb[:], bkc[:])
            nc.gpsimd.dma_start(bv_sb[:], bvr[:])

            # ---- phase B: projections + LV ----
            with (
                tc.tile_pool(name="projsb", bufs=1) as prs,
                tc.tile_pool(name="projps", bufs=4, space="PSUM") as prp,
                tc.tile_pool(name="vps", bufs=4, space="PSUM") as vpp,
            ):
                xt_sb = prs.tile([128, NDC, S], bf16)
                xt8_sb = prs.tile([128, NDC, S], fp8)
                wq_sb = prs.tile([128, NDC, CLOC], fp8)
                wk_sb = prs.tile([128, NDC, CLOC], fp8)
                wv_sb = prs.tile([128, NDC, CLOC], bf16)
                # DMA issue occupies the issuing engine ~0.6ns/descriptor+
                # ~5ns/KB, so spread the 8MB of loads across the three
                # DMA-capable engines, in consumption order per engine.
                for dc in range(0, NDC, 2):  # QK inputs first, dc-granular
                    nc.scalar.dma_start(
                        wq_sb[:, dc:dc + 2, :],
                        wq8[dc * 128:(dc + 2) * 128, :].rearrange(
                            "(two p) c -> p two c", p=128))
                    nc.scalar.dma_start(xt8_sb[:, dc, :],
                                        xt8[dc * 128:(dc + 1) * 128, :])
                    nc.sync.dma_start(xt8_sb[:, dc + 1, :],
                                      xt8[(dc + 1) * 128:(dc + 2) * 128, :])
                    nc.sync.dma_start(
                        wk_sb[:, dc:dc + 2, :],
                        wk8[dc * 128:(dc + 2) * 128, :].rearrange(
                            "(two p) c -> p two c", p=128))
                for dc in range(0, NDC, 2):
                    nc.sync.dma_start(xt_sb[:, dc, :],
                                      xt[dc * 128:(dc + 1) * 128, :])
                for dc in range(1, NDC, 2):
                    nc.gpsimd.dma_start(xt_sb[:, dc, :],
                                        xt[dc * 128:(dc + 1) * 128, :])
                for dc in range(NDC):
                    nc.sync.dma_start(wv_sb[:, dc, :],
                                      wvt[dc * 128:(dc + 1) * 128, :])
                nc.gpsimd.dma_start(wot_sb[:, :, :],
                                    wot.rearrange("(cc p) o -> p cc o", p=128))
                nc.gpsimd.dma_start(lt_sb[:, :, :],
                                    ltt.rearrange("u p q -> p u q"))

                # QT/KT: [c-tile 128, s-block 512], fp8 DoubleRow matmuls
                # contract 2 d-chunks per instruction at 2x rate. Weights are
                # pre-scaled x16 on host (fp8e4m3 normal range); the combined
                # 1/(16*16*8) lands in the exp's scale. Bias (x16) is added
                # on the PSUM->SBUF copy (ScalarE, per-partition bias).
                for ct in range(2):
                    for dst, w_sb, b_sb in ((qt[ct], wq_sb, bq_sb),
                                            (kt[ct], wk_sb, bk_sb)):
                        pss = [prp.tile([128, QB], f32, tag="projps",
                                        name=f"pjps{qb}") for qb in range(NQB)]
                        for dcp in range(NDC // 2):
                            for qb in range(NQB):
                                nc.tensor.matmul(
                                    pss[qb][:],
                                    w_sb[:, 2 * dcp:2 * dcp + 2,
                                         ct * 128:(ct + 1) * 128],
                                    xt8_sb[:, 2 * dcp:2 * dcp + 2,
                                           qb * QB:(qb + 1) * QB],
                                    start=(dcp == 0), stop=(dcp == NDC // 2 - 1),
                                    perf_mode=DR)
                        for qb in range(NQB):
                            nc.scalar.activation(
                                dst[:, qb * QB:(qb + 1) * QB], pss[qb][:],
                                Ident, bias=b_sb[:, ct:ct + 1])

                # V natural: [s-chunk 128, 256], contraction over d
                for sc in range(NKC):
                    ps = vpp.tile([128, CLOC], f32, tag="vps")
                    for dc in range(NDC):
                        nc.tensor.matmul(
                            ps[:],
                            xt_sb[:, dc, sc * 128:(sc + 1) * 128],
                            wv_sb[:, dc, :],
                            start=(dc == 0), stop=False)
                    nc.tensor.matmul(ps[:], ones_c[:], bv_sb[:],
                                     start=False, stop=True)
                    nc.scalar.activation(
                        v_all[:, sc, :, 0:HD],
                        ps[:].rearrange("p (h e) -> p h e", h=NH),
                        mybir.ActivationFunctionType.Copy, scale=1.2)
                    nc.vector.tensor_copy(v_pair[:, sc, :], ps[:])



            # ---- phase C: attention, software-pipelined ----
            # Heads 2hp/2hp+1 live at partitions 0-63/64-127 of c-tile hp.
            # Per (qb, hp) group: scores(kc+2)/exp(kc+1) run ahead of pv(kc)
            # so ScalarE (exp, the bottleneck) streams back-to-back while PE
            # fills its spare cycles with injected out-projection matmuls.
            with (
                tc.tile_pool(name="pt", bufs=6) as ptp,
                tc.tile_pool(name="stage", bufs=4) as sp,
                tc.tile_pool(name="osb", bufs=4) as ob,
                tc.tile_pool(name="stps", bufs=2, space="PSUM") as stp,
                tc.tile_pool(name="ctxps", bufs=3, space="PSUM") as ctp,
            ):
                def emit_outproj(sc, ot, pool, sbpool):
                    ps = pool.tile([128, QB], f32, tag="ops")
                    # cc=1 first: hp1's ctxt half is blended later, so the
                    # scheduler cannot hoist this pair to a point where it
                    # would stall PE waiting on the hp0 stg DMA.
                    for cc in (1, 0):
                        nc.tensor.matmul(
                            ps[:],
                            ctxt_q[sc // 4][:, cc,
                                            (sc % 4) * 128:(sc % 4 + 1) * 128],
                            wot_sb[:, cc, ot * QB:(ot + 1) * QB],
                            start=(cc == 1), stop=(cc == 0),
                            skip_group_check=True)
                    o_sb = sbpool.tile([128, QB], f32, tag="osb")
                    nc.vector.tensor_copy(o_sb[:], ps[:])
                    nc.sync.dma_start(
                        out[sc * 128:(sc + 1) * 128, ot * QB:(ot + 1) * QB],
                        o_sb[:])

                def lv_closures(lvp):
                    # banded 0.4*L^T @ V, both heads per matmul (M=128),
                    # chopped into per-matmul closures injected into the
                    # first two groups' kc loops as PE filler.
                    items = []
                    for qb, hp in groups:
                        slots = _LT_SLOTS[qb]
                        cell = {}

                        def mk(n, j, u, qb, hp, cell, last):
                            def go():
                                if n == 0:
                                    cell['ps'] = lvp.tile(
                                        [128, QB], f32, tag="lvps",
                                        name="lv_ps")
                                nc.tensor.matmul(
                                    cell['ps'][:],
                                    v_pair[:, j, hp * 128:(hp + 1) * 128],
                                    lt_sb[:, u, :],
                                    start=(n == 0), stop=last,
                                    skip_group_check=True)
                                if last:
                                    nc.vector.tensor_copy(
                                        ctxt_q[qb][:, hp, :], cell['ps'][:])
                            return go
                        for n, (j, u) in enumerate(slots):
                            items.append(mk(n, j, u, qb, hp, cell,
                                            n == len(slots) - 1))
                    return items

                filler = []

                def mk_blend(qb, hp, css):
                    # blend: ctxt (pre-loaded with 0.4LV) += PV * 0.6/r,
                    # running entirely off the SBUF cs copies. Deferred into
                    # the NEXT group's kc loop so nothing gates a boundary.
                    # ScalarE is untouched: the partition 64->0 move is a
                    # tiny SBUF-SBUF DMA; the adds are DMA accumulates.
                    qsl = slice(qb * QB, (qb + 1) * QB)

                    def blend():
                        for hh in range(2):
                            cs = css[hh]
                            bcs = sp.tile([1, QB], f32, tag="bcs")
                            nc.sync.dma_start(bcs[0:1, :], cs[64:65, :])
                            rc = sp.tile([1, QB], f32, tag="rc")
                            nc.vector.reciprocal_approx_fast(rc[:], bcs[:])
                            bc = sp.tile([64, QB], f32, tag="bc")
                            nc.gpsimd.partition_broadcast(
                                bc[:], rc[0:1, :], channels=HD)
                            m1 = sp.tile([64, QB], bf16, tag="m1")
                            nc.vector.tensor_mul(m1[:], cs[0:HD, :], bc[:])
                            if hh == 0:
                                nc.vector.tensor_add(
                                    ctxt_q[qb][0:64, hp, :], m1[:],
                                    ctxt_q[qb][0:64, hp, :])
                            else:
                                nc.gpsimd.dma_start(
                                    ctxt_q[qb][64:128, hp, :],
                                    m1[:], accum_op=add)
                    return blend

                def emit_group(qb, hp, prev_blend, pops):
                    qsl = slice(qb * QB, (qb + 1) * QB)
                    ctx = [ctp.tile([128, QB], f32, tag="ctxps",
                                    name=f"ctx{hh}") for hh in range(2)]
                    sts, pts = {}, {}

                    def scores(kc):
                        st = stp.tile([128, 2, QB], f32, tag="stps")
                        sts[kc] = st
                        for hh in range(2):
                            p0 = hh * 64
                            nc.tensor.matmul(
                                st[:, hh, :],
                                kt[hp][p0:p0 + 64, kc * 128:(kc + 1) * 128],
                                qt[hp][p0:p0 + 64, qsl],
                                start=True, stop=True,
                                tile_position=(p0, 0))
                        if kc % 2 == 0:
                            pts[kc // 2] = ptp.tile([128, 2, 2, QB], fp8,
                                                    tag="pt", name="ptt")
                        nc.scalar.activation(pts[kc // 2][:, kc % 2, :, :],
                                             st[:], Exp, scale=1.0 / 2048.0)

                    def pv_pair(kcp):
                        # fp8 DoubleRow: contract 2 k-chunks per instruction
                        ptt = pts.pop(kcp)
                        sts.pop(2 * kcp, None)
                        sts.pop(2 * kcp + 1, None)
                        for hh in range(2):
                            nc.tensor.matmul(
                                ctx[hh][0:HD + 4, :],
                                v_all[:, 2 * kcp:2 * kcp + 2,
                                      2 * hp + hh, 0:HD + 4],
                                ptt[:, :, hh, :],
                                start=(kcp == 0), stop=(kcp == NKC // 2 - 1),
                                perf_mode=DR)

                    scores(0)
                    scores(1)
                    for kc in range(NKC):
                        if kc + 2 < NKC:
                            scores(kc + 2)
                        if kc % 2 == 1:
                            pv_pair(kc // 2)
                        if kc == 1 and prev_blend is not None:
                            prev_blend()
                        for _ in range(pops(kc)):
                            if filler:
                                filler.pop(0)()
                    # copy ctx off PSUM right away: the banks free ~1us after
                    # pv(15), so the next group's accumulation never stalls
                    css = []
                    for hh in range(2):
                        cs = sp.tile([65, QB], f32, tag="cs")
                        nc.vector.tensor_copy(cs[:], ctx[hh][0:HD + 1, :])
                        css.append(cs)
                    return mk_blend(qb, hp, css)

                # (qb, hp) order chosen so out-proj(qb) becomes injectable
                # 1-2 groups after both hp halves of qb have blended.
                groups = [(0, 0), (0, 1), (1, 0), (1, 1),
                          (2, 0), (2, 1), (3, 0), (3, 1)]
                lv_pops = lambda kc: 2
                op_pops = lambda kc: 1 if kc in (3, 5, 8, 11, 14) else 0
                push_after = {2: 0, 3: 1, 5: 2}
                pending_blend = None
                # g0/g1: LV matmuls (PE filler) under a 1-bank PSUM pool that
                # closes before the out-projection pool opens.
                with tc.tile_pool(name="lvps", bufs=1, space="PSUM") as lvp:
                    filler.extend(lv_closures(lvp))
                    for gi in (0, 1):
                        qb, hp = groups[gi]
                        pending_blend = emit_group(qb, hp, pending_blend,
                                                   lv_pops)
                    while filler:  # safety: drain any LV leftovers
                        filler.pop(0)()
                opq = [(sc, ot) for qb_o in range(3)
                       for ot in range(2) for sc in range(qb_o * 4, qb_o * 4 + 4)]
                op_state = {"pushed": 0, "popped": 0}
                with tc.tile_pool(name="ops", bufs=1, space="PSUM") as opp:
                    def op_closure():
                        if op_state["popped"] < op_state["pushed"]:
                            sc, ot = opq[op_state["popped"]]
                            op_state["popped"] += 1
                            emit_outproj(sc, ot, opp, ob)
                    filler.extend([op_closure] * 64)  # pops gated by pushed
                    for gi in range(2, 8):
                        qb, hp = groups[gi]
                        pending_blend = emit_group(qb, hp, pending_blend,
                                                   op_pops)
                        if gi in push_after:
                            op_state["pushed"] = 8 * (push_after[gi] + 1)
                    pending_blend()
                    leftovers = opq[op_state["popped"]:]
                    filler.clear()

            # ---- phase D: out-projection tail (wider PSUM pool) ----
            with (
                tc.tile_pool(name="ops2", bufs=4, space="PSUM") as opp2,
                tc.tile_pool(name="osb2", bufs=4) as ob2,
            ):
                for sc, ot in leftovers:
                    emit_outproj(sc, ot, opp2, ob2)
                for sc in range(12, 16):
                    for ot in range(2):
                        emit_outproj(sc, ot, opp2, ob2)

    nc.compile()
    return nc


def _get_program():
    if "nc" not in _CACHE:
        _CACHE["nc"] = _build_program()
    return _CACHE["nc"]


def _in_maps(x, Wq, bq, Wk, bk, Wv, bv, Wo):
    FP8 = ml_dtypes.float8_e4m3fn
    xT = [np.ascontiguousarray(x[b].T).astype(BF16) for b in range(2)]
    xT8 = [np.ascontiguousarray(x[b].T).astype(FP8) for b in range(2)]
    maps = []
    for c in range(8):
        b, hg = c // 4, c % 4
        hs, he = hg * CLOC, (hg + 1) * CLOC
        maps.append({
            "xt": xT[b],
            "xt8": xT8[b],
            "wq8": np.ascontiguousarray(Wq[hs:he].T * F32(16.0)).astype(FP8),
            "wk8": np.ascontiguousarray(Wk[hs:he].T * F32(16.0)).astype(FP8),
            "wvt": np.ascontiguousarray(Wv[hs:he].T).astype(BF16),
            "bqc": np.ascontiguousarray((bq[hs:he] * F32(16.0)).reshape(2, 128).T),
            "bkc": np.ascontiguousarray((bk[hs:he] * F32(16.0)).reshape(2, 128).T),
            "bvr": bv[hs:he][None, :].astype(BF16),
            "wot": np.ascontiguousarray(Wo[:, hs:he].T).astype(BF16),
            "ltt": _LT_UNIQ,
        })
    return maps


def _run(x, Wq, bq, Wk, bk, Wv, bv, Wo, bo, trace=False):
    from concourse.bass_utils import run_bass_kernel_spmd
    nc = _get_program()
    maps = _in_maps(np.asarray(x, F32), np.asarray(Wq, F32), np.asarray(bq, F32),
                    np.asarray(Wk, F32), np.asarray(bk, F32), np.asarray(Wv, F32),
                    np.asarray(bv, F32), np.asarray(Wo, F32))
    res = run_bass_kernel_spmd(nc, maps, list(range(8)), trace=trace)
    bo = np.asarray(bo, F32)
    outp = np.empty((2, S, D), F32)
    for b in range(2):
        acc = res.results[b * 4]["out"].astype(F32)
        for hg in range(1, 4):
            acc = acc + res.results[b * 4 + hg]["out"]
        outp[b] = acc + bo
    return outp, res


def kernel(x, Wq, bq, Wk, bk, Wv, bv, Wo, bo):
    outp, _ = _run(x, Wq, bq, Wk, bk, Wv, bv, Wo, bo, trace=False)
    return outp


def kernel_traced(**inputs):
    return _run(trace=True, **inputs)
